# revision 1
# baseline (speedup 1.0000x reference)
"""Deformable conv2d + residual add + ReLU on 8 Trainium2 NeuronCores.

Self-contained harness entry: kernel(**inputs) -> np.ndarray.
Sharding: data-parallel over batch N=8 (one image per core); weight/bias
replicated. Each core runs the same Bass/Tile program.

Per-core pipeline:
  A) sampling indices + bilinear weights from offsets (DVE + PE transposes)
  B) zero-padded image and difference planes [x, Dx, Dy, Dxy] transposed to
     q-major 2KB rows in DRAM
  C) per kernel-tap k: SWDGE dma_gather of sample rows (position-major),
     bilinear combine with fused scalar_tensor_tensor ops (per-partition
     scalars), PE transpose back to channel-major, matmul accumulated in PSUM
  D) epilogue: + x2 + bias, ReLU, store.

Math: bilinear(x, py, px) = x[q] + wx*Dx[q] + wy*Dy[q] + wx*wy*Dxy[q] with
q = (floor(py)+PD)*WPAD + (floor(px)+PD) on the zero-padded grid; the zero
padding reproduces torchvision's out-of-bounds zeroing exactly, and clamping
floor() into the pad ring keeps fully-out-of-range samples at zero.
"""

import sys

for _p in ("/opt/trn_rl_repo",):
    if _p not in sys.path:
        sys.path.insert(0, _p)

import numpy as np

import concourse.bacc as bacc
import concourse.mybir as mybir
import concourse.tile as tile
from concourse import bass_utils
from concourse.masks import make_identity

F32 = mybir.dt.float32
I32 = mybir.dt.int32
I16 = mybir.dt.int16
A = mybir.AluOpType

# problem constants (nn_DeformConvAddReLU2d: N=8, C=Cout=128, 56x56, 3x3)
N, C, H, W = 8, 128, 56, 56
K = 9
PD = 2
HP, WP = H + 2 * PD, W + 2 * PD          # 60, 60
Q = HP * WP                               # 3600
NPOS = H * W                              # 3136
NPB = 3200                                # samples per tap padded to 25 blocks
NBLK = NPB // 128                         # 25
ELEM = 512                                # gathered row: [x|Dx|Dy|Dxy] x 128c
SLOTS = NPB // 16                         # 200 wrapped idx slots per tap


def host_consts():
    ki = np.arange(3).repeat(3)
    kj = np.tile(np.arange(3), 3)
    i = np.arange(H)
    j = np.arange(W)
    by = (i[None, :, None] - 1 + ki[:, None, None]).astype(np.float32)
    bx = (j[None, None, :] - 1 + kj[:, None, None]).astype(np.float32)
    by = np.broadcast_to(by, (K, H, W)).reshape(K, NPOS).copy()
    bx = np.broadcast_to(bx, (K, H, W)).reshape(K, NPOS).copy()
    return by, bx


def build_kernel(tc, outs, ins):
    nc = tc.nc
    out_d = outs                                   # [128, NPOS] f32
    x_d, off_d, x2_d, wt_d, bias_d, by_d, bx_d = ins

    with tc.tile_pool(name="persist", bufs=1) as pers, \
         tc.tile_pool(name="dram", bufs=1, space="DRAM") as dp:
        g4r = dp.tile([Q, ELEM], F32)
        qfs_d = dp.tile([K, NPB], F32)

        idn = pers.tile([128, 128], F32)
        make_identity(nc, idn[:])
        wsc = pers.tile([128, NBLK, 18], F32)      # scalars: wx at k, wy at 9+k
        nc.vector.memset(wsc[:], 0.0)
        idxw = pers.tile([128, K * SLOTS], I16)    # wrapped gather indices
        w_sb = pers.tile([128, K * 128], F32)      # lhsT per tap: [c, o]
        bias_sb = pers.tile([128, 1], F32)
        x2_sb = pers.tile([128, NPOS], F32)

        nc.sync.dma_start(out=w_sb[:], in_=wt_d[:])
        nc.sync.dma_start(out=bias_sb[:], in_=bias_d[:])
        nc.sync.dma_start(out=x2_sb[:], in_=x2_d[:])

        # ---------------- Phase 1: indices + weights ----------------
        with tc.tile_pool(name="idxsb", bufs=1) as sp, \
             tc.tile_pool(name="idxps", bufs=2, space="PSUM") as pp:
            wx_sb = sp.tile([9, NPOS], F32, tag="wxp")
            wy_sb = sp.tile([9, NPOS], F32, tag="wyp")
            qf = sp.tile([9, NPB], F32, tag="qf")
            nc.vector.memset(qf[:], 0.0)

            offv = off_d[:].rearrange("(k two) q -> two k q", two=2)
            for axis in range(2):  # 0: y, 1: x
                dv = sp.tile([9, NPOS], F32, tag="dv")
                nc.sync.dma_start(out=dv[:], in_=offv[axis])
                base = sp.tile([9, NPOS], F32, tag="base")
                nc.sync.dma_start(out=base[:], in_=(by_d if axis == 0 else bx_d)[:])
                p = sp.tile([9, NPOS], F32, tag="p")
                nc.vector.tensor_tensor(out=p[:], in0=dv[:], in1=base[:], op=A.add)
                tr = sp.tile([9, NPOS], F32, tag="tr")
                nc.vector.tensor_scalar(out=tr[:], in0=p[:], scalar1=64.0,
                                        scalar2=None, op0=A.add)
                tcl = sp.tile([9, NPOS], F32, tag="tcl")
                nc.vector.tensor_scalar(out=tcl[:], in0=tr[:], scalar1=62.0,
                                        scalar2=120.0, op0=A.max, op1=A.min)
                ri = sp.tile([9, NPOS], I32, tag="ri")
                nc.vector.tensor_copy(out=ri[:], in_=tcl[:])
                rf = sp.tile([9, NPOS], F32, tag="rf")
                nc.vector.tensor_copy(out=rf[:], in_=ri[:])
                gtt = sp.tile([9, NPOS], F32, tag="gtt")
                nc.vector.tensor_tensor(out=gtt[:], in0=rf[:], in1=tcl[:], op=A.is_gt)
                fl = sp.tile([9, NPOS], F32, tag="fl")      # floor(clamped)+64
                nc.vector.tensor_tensor(out=fl[:], in0=rf[:], in1=gtt[:],
                                        op=A.subtract)
                wdst = wy_sb if axis == 0 else wx_sb
                nc.vector.tensor_tensor(out=wdst[:], in0=tr[:], in1=fl[:],
                                        op=A.subtract)
                if axis == 0:
                    nc.vector.tensor_scalar(out=qf[:, :NPOS], in0=fl[:], scalar1=60.0,
                                            scalar2=-3782.0, op0=A.mult, op1=A.add)
                else:
                    nc.vector.tensor_tensor(out=qf[:, :NPOS], in0=qf[:, :NPOS],
                                            in1=fl[:], op=A.add)

            nc.sync.dma_start(out=qfs_d[:], in_=qf[:])
            qw = sp.tile([16, K * SLOTS], F32, tag="qw")
            nc.sync.dma_start(out=qw[:],
                              in_=qfs_d[:].rearrange("k (s p) -> p (k s)", p=16))
            nc.vector.tensor_copy(out=idxw[:16, :], in_=qw[:])
            for r in range(1, 8):
                nc.sync.dma_start(out=idxw[16 * r:16 * (r + 1), :], in_=idxw[0:16, :])

            for b in range(NBLK):
                n = min(128, NPOS - b * 128)
                if n <= 0:
                    break
                ptw = pp.tile([128, 32], F32)
                nc.tensor.transpose(out=ptw[:n, 0:9],
                                    in_=wx_sb[:, b * 128:b * 128 + n],
                                    identity=idn[:9, :9])
                nc.tensor.transpose(out=ptw[:n, 9:18],
                                    in_=wy_sb[:, b * 128:b * 128 + n],
                                    identity=idn[:9, :9])
                nc.scalar.copy(out=wsc[:n, b, :], in_=ptw[:n, 0:18])

        # ---------------- Phase 2: padded planes -> q-major rows ----------------
        with tc.tile_pool(name="plsb", bufs=1) as sp, \
             tc.tile_pool(name="plev", bufs=3) as evp, \
             tc.tile_pool(name="plps", bufs=2, space="PSUM") as pp:
            xp = sp.tile([128, Q], F32)
            nc.vector.memset(xp[:], 0.0)
            xpv = xp[:].rearrange("c (h w) -> c h w", h=HP)
            nc.sync.dma_start(out=xpv[:, PD:PD + H, PD:PD + W],
                              in_=x_d[:].rearrange("c (h w) -> c h w", h=H))
            dx = sp.tile([128, Q], F32)
            dy = sp.tile([128, Q], F32)
            dxy = sp.tile([128, Q], F32)
            nc.vector.memset(dx[:, Q - 1:], 0.0)
            nc.vector.memset(dy[:, Q - WP:], 0.0)
            nc.vector.memset(dxy[:, Q - WP - 1:], 0.0)
            nc.vector.tensor_tensor(out=dx[:, :Q - 1], in0=xp[:, 1:Q],
                                    in1=xp[:, :Q - 1], op=A.subtract)
            nc.vector.tensor_tensor(out=dy[:, :Q - WP], in0=xp[:, WP:Q],
                                    in1=xp[:, :Q - WP], op=A.subtract)
            nc.vector.tensor_tensor(out=dxy[:, :Q - WP - 1], in0=dx[:, WP:Q - 1],
                                    in1=dx[:, :Q - WP - 1], op=A.subtract)
            planes = [xp, dx, dy, dxy]
            for b in range(29):
                n = min(128, Q - b * 128)
                pt = pp.tile([128, ELEM], F32)
                for t, pl in enumerate(planes):
                    nc.tensor.transpose(out=pt[:n, 128 * t:128 * (t + 1)],
                                        in_=pl[:, b * 128:b * 128 + n],
                                        identity=idn[:])
                ev = evp.tile([128, ELEM], F32)
                nc.scalar.copy(out=ev[:n], in_=pt[:n])
                nc.sync.dma_start(out=g4r[b * 128:b * 128 + n, :], in_=ev[:n])

        # ---------------- Phase 3: gather / combine / matmul ----------------
        with tc.tile_pool(name="gk", bufs=2) as gp, \
             tc.tile_pool(name="cp", bufs=2) as cpp, \
             tc.tile_pool(name="cols", bufs=2) as csp, \
             tc.tile_pool(name="uv", bufs=3) as uvp, \
             tc.tile_pool(name="accp", bufs=1, space="PSUM") as accp, \
             tc.tile_pool(name="tps", bufs=1, space="PSUM") as tpp:
            acc = accp.tile([128, NPOS], F32)
            halves = [(0, 8), (8, 8), (16, 8), (24, 1)]   # (start block, nblocks)
            for k in range(K):
                colsP = cpp.tile([128, NPB], F32)          # pos-major combined
                for hb, nb in halves:
                    gk = gp.tile([128, 8, ELEM], F32)
                    nc.gpsimd.dma_gather(
                        gk[:, :nb, :], g4r[:],
                        idxw[:, k * SLOTS + hb * 8: k * SLOTS + hb * 8 + nb * 8],
                        num_idxs=nb * 128, num_idxs_reg=nb * 128, elem_size=ELEM)
                    for j in range(nb):
                        b = hb + j
                        g0 = gk[:, j, 0:128]
                        g1 = gk[:, j, 128:256]
                        g2 = gk[:, j, 256:384]
                        g3 = gk[:, j, 384:512]
                        wxs = wsc[:, b, k:k + 1]
                        wys = wsc[:, b, 9 + k:10 + k]
                        u = uvp.tile([128, 128], F32, tag="u")
                        nc.vector.scalar_tensor_tensor(u[:], g3, wxs, g2,
                                                       op0=A.mult, op1=A.add)
                        v = uvp.tile([128, 128], F32, tag="v")
                        nc.vector.scalar_tensor_tensor(v[:], g1, wxs, g0,
                                                       op0=A.mult, op1=A.add)
                        nc.vector.scalar_tensor_tensor(
                            colsP[:, b * 128:(b + 1) * 128], u[:], wys, v[:],
                            op0=A.mult, op1=A.add)
                cols = csp.tile([128, NPB], F32)           # c-major
                for g in range(7):
                    bs = list(range(4 * g, min(4 * g + 4, NBLK)))
                    ptc = tpp.tile([128, 512], F32)
                    for j, b in enumerate(bs):
                        nc.tensor.transpose(out=ptc[:, 128 * j:128 * (j + 1)],
                                            in_=colsP[:, b * 128:(b + 1) * 128],
                                            identity=idn[:])
                    wdt = len(bs) * 128
                    nc.scalar.copy(out=cols[:, 512 * g:512 * g + wdt],
                                   in_=ptc[:, :wdt])
                for ch in range(7):
                    lo = 512 * ch
                    hi = min(lo + 512, NPOS)
                    nc.tensor.matmul(acc[:, lo:hi],
                                     lhsT=w_sb[:, k * 128:(k + 1) * 128],
                                     rhs=cols[:, lo:hi],
                                     start=(k == 0), stop=(k == K - 1))

            # ---------------- epilogue ----------------
            tmp = csp.tile([128, NPOS], F32, tag="epi1")
            nc.vector.tensor_tensor(out=tmp[:], in0=acc[:], in1=x2_sb[:], op=A.add)
            outp = cpp.tile([128, NPOS], F32, tag="epi2")
            nc.scalar.activation(outp[:], tmp[:], mybir.ActivationFunctionType.Relu,
                                 bias=bias_sb[:], scale=1.0)
            nc.sync.dma_start(out=out_d[:], in_=outp[:])


def make_core_inputs(x, offset, weight, bias, x2):
    """Full inputs -> list of 8 per-core input dicts (host batch sharding)."""
    by, bx = host_consts()
    wt = np.ascontiguousarray(
        weight.reshape(128, 128, K).transpose(1, 2, 0).reshape(128, K * 128)
    ).astype(np.float32)
    cores = []
    for i in range(N):
        cores.append({
            "x": np.ascontiguousarray(x[i].reshape(C, NPOS), dtype=np.float32),
            "off": np.ascontiguousarray(offset[i].reshape(2 * K, NPOS),
                                        dtype=np.float32),
            "x2": np.ascontiguousarray(x2[i].reshape(C, NPOS), dtype=np.float32),
            "wt": wt,
            "bias": np.ascontiguousarray(bias.reshape(C, 1), dtype=np.float32),
            "by": by,
            "bx": bx,
        })
    return cores


_CACHED_NC = None

IN_SPECS = [("x", (C, NPOS)), ("off", (2 * K, NPOS)), ("x2", (C, NPOS)),
            ("wt", (C, K * 128)), ("bias", (C, 1)), ("by", (K, NPOS)),
            ("bx", (K, NPOS))]


def _build_nc():
    global _CACHED_NC
    if _CACHED_NC is not None:
        return _CACHED_NC
    nc = bacc.Bacc("TRN2", target_bir_lowering=False, debug=False, num_devices=N)
    ins = [nc.dram_tensor(nm, list(sh), F32, kind="ExternalInput").ap()
           for nm, sh in IN_SPECS]
    out = nc.dram_tensor("out", [C, NPOS], F32, kind="ExternalOutput").ap()
    with tile.TileContext(nc, trace_sim=False) as tc:
        build_kernel(tc, out, ins)
    nc.compile()
    _CACHED_NC = nc
    return nc


def run_cores(inputs, trace=False):
    """Run the SPMD kernel; returns (out [N,C,H,W] f32, exec_time_ns or None)."""
    nc = _build_nc()
    in_maps = make_core_inputs(inputs["x"], inputs["offset"], inputs["weight"],
                               inputs["bias"], inputs["x2"])
    res = bass_utils.run_bass_kernel_spmd(nc, in_maps, core_ids=list(range(N)),
                                          trace=trace)
    out = np.stack([res.results[i]["out"] for i in range(N)])
    return out.reshape(N, C, H, W), res.exec_time_ns


def kernel(x, offset, weight, bias, x2):
    x = np.asarray(x, dtype=np.float32)
    offset = np.asarray(offset, dtype=np.float32)
    weight = np.asarray(weight, dtype=np.float32)
    bias = np.asarray(bias, dtype=np.float32)
    x2 = np.asarray(x2, dtype=np.float32)
    out, _ = run_cores({"x": x, "offset": offset, "weight": weight,
                        "bias": bias, "x2": x2}, trace=False)
    return out



# revision 12
# speedup vs baseline: 1.4186x; 1.4186x over previous
"""Deformable conv2d + residual add + ReLU on 8 Trainium2 NeuronCores.

Self-contained harness entry: kernel(**inputs) -> np.ndarray.
Sharding: data-parallel over batch N=8 (one image per core); weight/bias
replicated. Each core runs the same Bass/Tile program.

v2 design (gather-descgen-bound pipeline, bf16 data path):
  A) index/weight chains from offsets on DVE. The gather-index chain runs
     directly in the SWDGE wrapped layout [16, (axis,k,slot)] (offsets are
     pre-wrapped on host), so idxw needs no transpose/round-trip. A second
     small chain in [126, 448] packed layout produces the bilinear weights,
     PE-transposed per 128-position block into per-partition scalars.
  B) zero-padded image planes [x, Dy, Dx, Dxy] in bf16, PE-transposed to
     q-major 1KB rows in DRAM.
  C) per kernel-tap k: ONE SWDGE dma_gather of 3200 sample rows
     (position-major), bilinear combine with 2 fused scalar_tensor_tensor
     ops per block (plane pairing [x|Dy]+wx*[Dx|Dxy], then +wy*hi), PE
     transpose back to channel-major, bf16 matmul accumulated in PSUM.
  D) epilogue: + x2 + bias, ReLU, store f32.

Math: bilinear(x, py, px) = x[q] + wx*Dx[q] + wy*Dy[q] + wx*wy*Dxy[q] with
q = floor(py+PD)*WP + floor(px+PD) on the zero-padded grid; the zero
padding reproduces torchvision's out-of-bounds zeroing exactly, and clamping
floor() into the pad ring keeps fully-out-of-range samples at zero.
"""

import sys

for _p in ("/opt/trn_rl_repo",):
    if _p not in sys.path:
        sys.path.insert(0, _p)

import numpy as np
import ml_dtypes

import concourse.bacc as bacc
import concourse.mybir as mybir
import concourse.tile as tile
from concourse import bass_utils
from concourse.masks import make_identity

F32 = mybir.dt.float32
BF16 = mybir.dt.bfloat16
I32 = mybir.dt.int32
I16 = mybir.dt.int16
A = mybir.AluOpType

# problem constants (nn_DeformConvAddReLU2d: N=8, C=Cout=128, 56x56, 3x3)
N, C, H, W = 8, 128, 56, 56
K = 9
PD = 2
HP, WP = H + 2 * PD, W + 2 * PD          # 60, 60
Q = HP * WP                               # 3600
NPOS = H * W                              # 3136
NPB = 3200                                # samples per tap padded to 25 blocks
NBLK = NPB // 128                         # 25
ELEM = 512                                # row: [x|Dy|Dx|Dxy] x 128c bf16 (1KB)
SLOTS = NPB // 16                         # 200 wrapped idx slots per tap
SPT = 7                                   # 3136 = 7 * 448 partition packing
FREE1 = NPOS // SPT                       # 448


def host_consts():
    """Base sampling positions, pre-biased by +PD (padded-grid coords).

    Returns:
      based: [126, 448] f32 — deinterleaved packed layout (axis, k, s) x f
             for the weight chain.
      basew: [16, 2*K*SLOTS] f32 — SWDGE-wrapped layout p x (axis, k, slot)
             for the gather-index chain; padded tail positions get -1000 so
             they clamp to q=0 (a guaranteed-zero pad row).
    """
    ki = np.arange(3).repeat(3)
    kj = np.tile(np.arange(3), 3)
    i = np.arange(H)
    j = np.arange(W)
    by = (i[None, :, None] + ki[:, None, None] + 1).astype(np.float32)
    bx = (j[None, None, :] + kj[:, None, None] + 1).astype(np.float32)
    by = np.broadcast_to(by, (K, H, W)).reshape(K, NPOS)
    bx = np.broadcast_to(bx, (K, H, W)).reshape(K, NPOS)
    based = np.concatenate(
        [by.reshape(K * SPT, FREE1), bx.reshape(K * SPT, FREE1)], axis=0
    ).astype(np.float32)

    byp = np.full((K, NPB), -1000.0, dtype=np.float32)
    bxp = np.full((K, NPB), -1000.0, dtype=np.float32)
    byp[:, :NPOS] = by
    bxp[:, :NPOS] = bx
    # wrap: [K, SLOTS, 16] -> [16, K, SLOTS]
    byw = byp.reshape(K, SLOTS, 16).transpose(2, 0, 1).reshape(16, K * SLOTS)
    bxw = bxp.reshape(K, SLOTS, 16).transpose(2, 0, 1).reshape(16, K * SLOTS)
    basew = np.concatenate([byw, bxw], axis=1)
    return based, np.ascontiguousarray(basew)


def wrap_offsets(off):
    """off [2K, NPOS] f32 -> SWDGE-wrapped [16, 2*K*SLOTS] (axis, k, slot)."""
    offp = np.zeros((2 * K, NPB), dtype=np.float32)
    offp[:, :NPOS] = off
    w = offp.reshape(K, 2, SLOTS, 16).transpose(3, 1, 0, 2)  # [16, 2, K, SLOTS]
    return np.ascontiguousarray(w.reshape(16, 2 * K * SLOTS))


def build_kernel(tc, outs, ins):
    nc = tc.nc
    out_d = outs                                   # [128, NPOS] f32
    x_d, offd_d, offw_d, x2_d, wt_d, bias_d, based_d, basew_d = ins

    with tc.tile_pool(name="persist", bufs=1) as pers, \
         tc.tile_pool(name="dram", bufs=1, space="DRAM") as dp:
        g4r = dp.tile([Q, ELEM], BF16)
        wd = dp.tile([126, FREE1], F32)

        idn = pers.tile([128, 128], F32)
        make_identity(nc, idn[:])
        idnb = pers.tile([128, 128], BF16)
        nc.vector.tensor_copy(out=idnb[:], in_=idn[:])
        wsc = pers.tile([128, NBLK, 18], BF16)     # scalars: wy at k, wx at 9+k
        nc.vector.memset(wsc[:], 0.0)
        idxw = pers.tile([128, K * SLOTS], I16)    # wrapped gather indices
        w_sb = pers.tile([128, K * 128], BF16)     # lhsT per tap: [c, o]
        bias_sb = pers.tile([128, 1], F32)
        x2_sb = pers.tile([128, NPOS], F32)

        nc.sync.dma_start(out=w_sb[:], in_=wt_d[:])
        nc.sync.dma_start(out=bias_sb[:], in_=bias_d[:])
        nc.sync.dma_start(out=x2_sb[:], in_=x2_d[:])

        # ---------------- Phase 1a: gather indices (wrapped layout) --------
        with tc.tile_pool(name="idxsb", bufs=1) as sp, \
             tc.tile_pool(name="idxps", bufs=2, space="PSUM") as pp:
            M = 2 * K * SLOTS                      # 3600
            dvw = sp.tile([16, M], F32, tag="dvw")
            nc.sync.dma_start(out=dvw[:], in_=offw_d[:])
            bw = sp.tile([16, M], F32, tag="bw")
            nc.sync.dma_start(out=bw[:], in_=basew_d[:])
            trw = sp.tile([16, M], F32, tag="trw")
            nc.vector.tensor_tensor(out=trw[:], in0=dvw[:], in1=bw[:], op=A.add)
            tclw = sp.tile([16, M], F32, tag="tclw")
            nc.vector.tensor_scalar(out=tclw[:], in0=trw[:], scalar1=0.0,
                                    scalar2=58.0, op0=A.max, op1=A.min)
            riw = sp.tile([16, M], I32, tag="riw")
            nc.vector.tensor_copy(out=riw[:], in_=tclw[:])
            rfw = sp.tile([16, M], F32, tag="rfw")
            nc.vector.tensor_copy(out=rfw[:], in_=riw[:])
            gttw = sp.tile([16, M], F32, tag="gttw")
            nc.vector.tensor_tensor(out=gttw[:], in0=rfw[:], in1=tclw[:],
                                    op=A.is_gt)
            flw = sp.tile([16, M], F32, tag="flw")
            nc.vector.tensor_tensor(out=flw[:], in0=rfw[:], in1=gttw[:],
                                    op=A.subtract)
            qfw = sp.tile([16, K * SLOTS], F32, tag="qfw")
            nc.vector.scalar_tensor_tensor(
                out=qfw[:], in0=flw[:, :K * SLOTS], scalar=float(WP),
                in1=flw[:, K * SLOTS:], op0=A.mult, op1=A.add)
            nc.vector.tensor_copy(out=idxw[:16, :], in_=qfw[:])
            for r in (16, 32, 64):
                nc.sync.dma_start(out=idxw[r:2 * r, :], in_=idxw[0:r, :])

            # ------------ Phase 1b: bilinear weights (packed layout) -------
            dv = sp.tile([126, FREE1], F32, tag="dv")
            nc.sync.dma_start(out=dv[:], in_=offd_d[:])
            bs = sp.tile([126, FREE1], F32, tag="bs")
            nc.sync.dma_start(out=bs[:], in_=based_d[:])
            tr = sp.tile([126, FREE1], F32, tag="tr")
            nc.vector.tensor_tensor(out=tr[:], in0=dv[:], in1=bs[:], op=A.add)
            tcl = sp.tile([126, FREE1], F32, tag="tcl")
            nc.vector.tensor_scalar(out=tcl[:], in0=tr[:], scalar1=0.0,
                                    scalar2=58.0, op0=A.max, op1=A.min)
            ri = sp.tile([126, FREE1], I32, tag="ri")
            nc.vector.tensor_copy(out=ri[:], in_=tcl[:])
            rf = sp.tile([126, FREE1], F32, tag="rf")
            nc.vector.tensor_copy(out=rf[:], in_=ri[:])
            gtt = sp.tile([126, FREE1], F32, tag="gtt")
            nc.vector.tensor_tensor(out=gtt[:], in0=rf[:], in1=tcl[:],
                                    op=A.is_gt)
            fl = sp.tile([126, FREE1], F32, tag="fl")
            nc.vector.tensor_tensor(out=fl[:], in0=rf[:], in1=gtt[:],
                                    op=A.subtract)
            wv = sp.tile([126, FREE1], F32, tag="wv")    # wy | wx
            nc.vector.tensor_tensor(out=wv[:], in0=tr[:], in1=fl[:],
                                    op=A.subtract)
            # reshuffle [126, 448] (a,k,s)xf -> [18, 3136] (a,k)x(s,f)
            # via DRAM (cross partition/free regrouping needs a flat hop)
            nc.sync.dma_start(out=wd[:], in_=wv[:])
            wsb2 = sp.tile([18, NPOS], F32, tag="wsb2")
            nc.sync.dma_start(
                out=wsb2[:],
                in_=wd[:].rearrange("(c s) f -> c (s f)", s=SPT))
            for b in range(NBLK):
                n = min(128, NPOS - b * 128)
                if n <= 0:
                    break
                ptw = pp.tile([128, 32], F32)
                nc.tensor.transpose(out=ptw[:n, 0:18],
                                    in_=wsb2[:, b * 128:b * 128 + n],
                                    identity=idn[:18, :18])
                nc.scalar.copy(out=wsc[:n, b, :], in_=ptw[:n, 0:18])

        # ---------------- Phase 2: padded planes -> q-major bf16 rows ------
        with tc.tile_pool(name="plsb", bufs=1) as sp, \
             tc.tile_pool(name="plev", bufs=3) as evp, \
             tc.tile_pool(name="plps", bufs=2, space="PSUM") as pp:
            xp = sp.tile([128, Q], F32)
            nc.vector.memset(xp[:], 0.0)
            xpv = xp[:].rearrange("c (h w) -> c h w", h=HP)
            nc.sync.dma_start(out=xpv[:, PD:PD + H, PD:PD + W],
                              in_=x_d[:].rearrange("c (h w) -> c h w", h=H))
            dxf = sp.tile([128, Q], F32)               # x[q+1]-x[q] in f32
            nc.vector.memset(dxf[:, Q - 1:], 0.0)
            nc.vector.tensor_tensor(out=dxf[:, :Q - 1], in0=xp[:, 1:Q],
                                    in1=xp[:, :Q - 1], op=A.subtract)
            xpb = sp.tile([128, Q], BF16)
            nc.vector.tensor_copy(out=xpb[:], in_=xp[:])
            dyb = sp.tile([128, Q], BF16)
            nc.vector.memset(dyb[:, Q - WP:], 0.0)
            nc.vector.tensor_tensor(out=dyb[:, :Q - WP], in0=xp[:, WP:Q],
                                    in1=xp[:, :Q - WP], op=A.subtract)
            dxb = sp.tile([128, Q], BF16)
            nc.vector.tensor_copy(out=dxb[:], in_=dxf[:])
            dxyb = sp.tile([128, Q], BF16)
            nc.vector.memset(dxyb[:, Q - WP - 1:], 0.0)
            nc.vector.tensor_tensor(out=dxyb[:, :Q - WP - 1],
                                    in0=dxf[:, WP:Q - 1],
                                    in1=dxf[:, :Q - WP - 1], op=A.subtract)
            planes = [xpb, dyb, dxb, dxyb]
            for b in range(29):
                n = min(128, Q - b * 128)
                pt = pp.tile([128, ELEM], BF16)
                for t, pl in enumerate(planes):
                    nc.tensor.transpose(out=pt[:n, 128 * t:128 * (t + 1)],
                                        in_=pl[:, b * 128:b * 128 + n],
                                        identity=idnb[:])
                ev = evp.tile([128, ELEM], BF16)
                nc.scalar.copy(out=ev[:n], in_=pt[:n])
                nc.sync.dma_start(out=g4r[b * 128:b * 128 + n, :], in_=ev[:n])

        # ---------------- Phase 3: gather / combine / matmul ----------------
        with tc.tile_pool(name="gk", bufs=2) as gp, \
             tc.tile_pool(name="cp", bufs=2) as cpp, \
             tc.tile_pool(name="cols", bufs=2) as csp, \
             tc.tile_pool(name="uv", bufs=4) as uvp, \
             tc.tile_pool(name="accp", bufs=1, space="PSUM") as accp, \
             tc.tile_pool(name="tps", bufs=1, space="PSUM") as tpp:
            acc = accp.tile([128, NPOS], F32)
            halves = [(0, 8), (8, 8), (16, 8), (24, 1)]   # (start block, nblocks)
            for k in range(K):
                gk = gp.tile([128, NBLK, ELEM], BF16)
                for hb, nb in halves:
                    nc.gpsimd.dma_gather(
                        gk[:, hb:hb + nb, :], g4r[:],
                        idxw[:, k * SLOTS + hb * 8:k * SLOTS + (hb + nb) * 8],
                        num_idxs=nb * 128, num_idxs_reg=nb * 128,
                        elem_size=ELEM)
                colsP = cpp.tile([128, NPB], BF16)     # pos-major combined
                for b in range(NBLK):
                    wys = wsc[:, b, k:k + 1]
                    wxs = wsc[:, b, 9 + k:10 + k]
                    uv = uvp.tile([128, 256], BF16, tag="uv")
                    # uv = [x|Dy] + wx*[Dx|Dxy]  ->  [v', u']
                    nc.vector.scalar_tensor_tensor(
                        uv[:], gk[:, b, 256:512], wxs, gk[:, b, 0:256],
                        op0=A.mult, op1=A.add)
                    # cols = v' + wy*u'
                    nc.vector.scalar_tensor_tensor(
                        colsP[:, b * 128:(b + 1) * 128], uv[:, 128:256], wys,
                        uv[:, 0:128], op0=A.mult, op1=A.add)
                cols = csp.tile([128, NPB], BF16)      # c-major
                for g in range(7):
                    bs_ = list(range(4 * g, min(4 * g + 4, NBLK)))
                    ptc = tpp.tile([128, 512], BF16)
                    for j, b in enumerate(bs_):
                        nc.tensor.transpose(out=ptc[:, 128 * j:128 * (j + 1)],
                                            in_=colsP[:, b * 128:(b + 1) * 128],
                                            identity=idnb[:])
                    wdt = len(bs_) * 128
                    nc.scalar.copy(out=cols[:, 512 * g:512 * g + wdt],
                                   in_=ptc[:, :wdt])
                for ch in range(7):
                    lo = 512 * ch
                    hi = min(lo + 512, NPOS)
                    nc.tensor.matmul(acc[:, lo:hi],
                                     lhsT=w_sb[:, k * 128:(k + 1) * 128],
                                     rhs=cols[:, lo:hi],
                                     start=(k == 0), stop=(k == K - 1))

            # ---------------- epilogue ----------------
            tmp = csp.tile([128, NPOS], F32, tag="epi1")
            nc.vector.tensor_tensor(out=tmp[:], in0=acc[:], in1=x2_sb[:], op=A.add)
            outp = cpp.tile([128, NPOS], F32, tag="epi2")
            nc.scalar.activation(outp[:], tmp[:], mybir.ActivationFunctionType.Relu,
                                 bias=bias_sb[:], scale=1.0)
            nc.sync.dma_start(out=out_d[:], in_=outp[:])


def make_core_inputs(x, offset, weight, bias, x2):
    """Full inputs -> list of 8 per-core input dicts (host batch sharding)."""
    based, basew = host_consts()
    wt = np.ascontiguousarray(
        weight.reshape(128, 128, K).transpose(1, 2, 0).reshape(128, K * 128)
    ).astype(ml_dtypes.bfloat16)
    cores = []
    for i in range(N):
        off = offset[i].reshape(2 * K, NPOS).astype(np.float32)
        offd = np.ascontiguousarray(
            off.reshape(K, 2, SPT, FREE1).transpose(1, 0, 2, 3)
            .reshape(2 * K * SPT, FREE1))
        cores.append({
            "x": np.ascontiguousarray(x[i].reshape(C, NPOS), dtype=np.float32),
            "offd": offd,
            "offw": wrap_offsets(off),
            "x2": np.ascontiguousarray(x2[i].reshape(C, NPOS), dtype=np.float32),
            "wt": wt,
            "bias": np.ascontiguousarray(bias.reshape(C, 1), dtype=np.float32),
            "based": based,
            "basew": basew,
        })
    return cores


_CACHED_NC = None

IN_SPECS = [("x", (C, NPOS), F32), ("offd", (2 * K * SPT, FREE1), F32),
            ("offw", (16, 2 * K * SLOTS), F32), ("x2", (C, NPOS), F32),
            ("wt", (C, K * 128), BF16), ("bias", (C, 1), F32),
            ("based", (2 * K * SPT, FREE1), F32),
            ("basew", (16, 2 * K * SLOTS), F32)]


def _build_nc():
    global _CACHED_NC
    if _CACHED_NC is not None:
        return _CACHED_NC
    nc = bacc.Bacc("TRN2", target_bir_lowering=False, debug=False, num_devices=N)
    ins = [nc.dram_tensor(nm, list(sh), dt, kind="ExternalInput").ap()
           for nm, sh, dt in IN_SPECS]
    out = nc.dram_tensor("out", [C, NPOS], F32, kind="ExternalOutput").ap()
    with tile.TileContext(nc, trace_sim=False) as tc:
        build_kernel(tc, out, ins)
    nc.compile()
    _CACHED_NC = nc
    return nc


def run_cores(inputs, trace=False):
    """Run the SPMD kernel; returns (out [N,C,H,W] f32, exec_time_ns or None)."""
    nc = _build_nc()
    in_maps = make_core_inputs(inputs["x"], inputs["offset"], inputs["weight"],
                               inputs["bias"], inputs["x2"])
    res = bass_utils.run_bass_kernel_spmd(nc, in_maps, core_ids=list(range(N)),
                                          trace=trace)
    out = np.stack([res.results[i]["out"] for i in range(N)])
    return out.reshape(N, C, H, W), res.exec_time_ns


def kernel(x, offset, weight, bias, x2):
    x = np.asarray(x, dtype=np.float32)
    offset = np.asarray(offset, dtype=np.float32)
    weight = np.asarray(weight, dtype=np.float32)
    bias = np.asarray(bias, dtype=np.float32)
    x2 = np.asarray(x2, dtype=np.float32)
    out, _ = run_cores({"x": x, "offset": offset, "weight": weight,
                        "bias": bias, "x2": x2}, trace=False)
    return out


# revision 18
# speedup vs baseline: 1.4220x; 1.0024x over previous
"""Deformable conv2d + residual add + ReLU on 8 Trainium2 NeuronCores.

Self-contained harness entry: kernel(**inputs) -> np.ndarray.
Sharding: data-parallel over batch N=8 (one image per core); weight/bias
replicated. Each core runs the same Bass/Tile program.

v3 design (gather-descgen-bound pipeline, bf16 data path):
  Prologue (phases overlap across engines):
  A) gather-index chain on DVE directly in the SWDGE wrapped layout
     [16, (axis,k,slot)] (offsets pre-wrapped on host) -> no transposes or
     scatter DMAs; floor via x - mod(x,1). A second small chain in packed
     [126, 448] layout produces the bilinear weights, PE-transposed per
     128-position block into per-partition scalars.
  B) zero-padded image planes [x, Dy, Dx, Dxy] in bf16 built on the GpSimd
     engine (parallel with A on DVE), PE-transposed to q-major 1KB rows in
     DRAM (two 128-blocks per PSUM eviction).
  Main loop, per kernel-tap k:
  C) SWDGE dma_gather of 3200 sample rows (position-major, 2 calls),
     bilinear combine with 2 fused scalar_tensor_tensor ops per block
     ([x|Dy] + wx*[Dx|Dxy], then + wy*hi), PE transpose back to
     channel-major, bf16 matmul accumulated in PSUM.
  D) epilogue: + x2 + bias, ReLU, store f32.

Math: bilinear(x, py, px) = x[q] + wx*Dx[q] + wy*Dy[q] + wx*wy*Dxy[q] with
q = floor(py+PD)*WP + floor(px+PD) on the zero-padded grid; the zero
padding reproduces torchvision's out-of-bounds zeroing exactly, and clamping
floor() into the pad ring keeps fully-out-of-range samples at zero.
"""

import sys

for _p in ("/opt/trn_rl_repo",):
    if _p not in sys.path:
        sys.path.insert(0, _p)

import numpy as np
import ml_dtypes

import concourse.bacc as bacc
import concourse.mybir as mybir
import concourse.tile as tile
from concourse import bass_utils
from concourse.masks import make_identity

F32 = mybir.dt.float32
BF16 = mybir.dt.bfloat16
I32 = mybir.dt.int32
I16 = mybir.dt.int16
A = mybir.AluOpType

# problem constants (nn_DeformConvAddReLU2d: N=8, C=Cout=128, 56x56, 3x3)
N, C, H, W = 8, 128, 56, 56
K = 9
PD = 2
HP, WP = H + 2 * PD, W + 2 * PD          # 60, 60
Q = HP * WP                               # 3600
NPOS = H * W                              # 3136
NPB = 3200                                # samples per tap padded to 25 blocks
NBLK = NPB // 128                         # 25
ELEM = 512                                # row: [x|Dy|Dx|Dxy] x 128c bf16 (1KB)
SLOTS = NPB // 16                         # 200 wrapped idx slots per tap
SPT = 7                                   # 3136 = 7 * 448 partition packing
FREE1 = NPOS // SPT                       # 448
GSPLIT = [(0, 8), (8, 8), (16, 8), (24, 1)]   # gather call split (start, nblocks)


def host_consts():
    """Base sampling positions, pre-biased by +PD (padded-grid coords).

    Returns:
      based: [126, 448] f32 — deinterleaved packed layout (axis, k, s) x f
             for the weight chain.
      basew: [16, 2*K*SLOTS] f32 — SWDGE-wrapped layout p x (axis, k, slot)
             for the gather-index chain; padded tail positions get -1000 so
             they clamp to q=0 (a guaranteed-zero pad row).
    """
    ki = np.arange(3).repeat(3)
    kj = np.tile(np.arange(3), 3)
    i = np.arange(H)
    j = np.arange(W)
    by = (i[None, :, None] + ki[:, None, None] + 1).astype(np.float32)
    bx = (j[None, None, :] + kj[:, None, None] + 1).astype(np.float32)
    by = np.broadcast_to(by, (K, H, W)).reshape(K, NPOS)
    bx = np.broadcast_to(bx, (K, H, W)).reshape(K, NPOS)
    based = np.concatenate(
        [by.reshape(K * SPT, FREE1), bx.reshape(K * SPT, FREE1)], axis=0
    ).astype(np.float32)

    byp = np.full((K, NPB), -1000.0, dtype=np.float32)
    bxp = np.full((K, NPB), -1000.0, dtype=np.float32)
    byp[:, :NPOS] = by
    bxp[:, :NPOS] = bx
    # wrap: [K, SLOTS, 16] -> [16, K, SLOTS]
    byw = byp.reshape(K, SLOTS, 16).transpose(2, 0, 1).reshape(16, K * SLOTS)
    bxw = bxp.reshape(K, SLOTS, 16).transpose(2, 0, 1).reshape(16, K * SLOTS)
    basew = np.concatenate([byw, bxw], axis=1)
    return based, np.ascontiguousarray(basew)


def wrap_offsets(off):
    """off [2K, NPOS] f32 -> SWDGE-wrapped [16, 2*K*SLOTS] (axis, k, slot)."""
    offp = np.zeros((2 * K, NPB), dtype=np.float32)
    offp[:, :NPOS] = off
    w = offp.reshape(K, 2, SLOTS, 16).transpose(3, 1, 0, 2)  # [16, 2, K, SLOTS]
    return np.ascontiguousarray(w.reshape(16, 2 * K * SLOTS))


def build_kernel(tc, outs, ins):
    nc = tc.nc
    out_d = outs                                   # [128, NPOS] f32
    x_d, offd_d, offw_d, x2_d, wt_d, bias_d, based_d, basew_d = ins

    with tc.tile_pool(name="persist", bufs=1) as pers, \
         tc.tile_pool(name="dram", bufs=1, space="DRAM") as dp:
        g4r = dp.tile([Q, ELEM], BF16)
        wd = dp.tile([126, FREE1], F32)

        idn = pers.tile([128, 128], F32)
        make_identity(nc, idn[:])
        idnb = pers.tile([128, 128], BF16)
        nc.vector.tensor_copy(out=idnb[:], in_=idn[:])
        wsc = pers.tile([128, NBLK, 18], BF16)     # scalars: wy at k, wx at 9+k
        nc.vector.memset(wsc[:], 0.0)
        idxw = pers.tile([128, K * SLOTS], I16)    # wrapped gather indices
        w_sb = pers.tile([128, K * 128], BF16)     # lhsT per tap: [c, o]
        bias_sb = pers.tile([128, 1], F32)
        x2_sb = pers.tile([128, NPOS], F32)

        nc.sync.dma_start(out=w_sb[:], in_=wt_d[:])
        nc.sync.dma_start(out=bias_sb[:], in_=bias_d[:])
        nc.sync.dma_start(out=x2_sb[:], in_=x2_d[:])

        # =============== Prologue: phases 1a/2/1b overlap ==================
        with tc.tile_pool(name="prosb", bufs=1) as sp, \
             tc.tile_pool(name="proev", bufs=3) as evp, \
             tc.tile_pool(name="props", bufs=2, space="PSUM") as pp, \
             tc.tile_pool(name="props2", bufs=2, space="PSUM") as pp2:
            # ---- Phase 1a: gather indices, wrapped layout, on DVE ----
            # 4 rotating [16, M] buffers (wa/wb/wc f32, wi i32) keep SBUF flat
            M = 2 * K * SLOTS                      # 3600
            wa = sp.tile([16, M], F32, tag="wa")
            wb = sp.tile([16, M], F32, tag="wb")
            wc = sp.tile([16, M], F32, tag="wc")
            wi = sp.tile([16, M], I32, tag="wi")
            nc.sync.dma_start(out=wa[:], in_=offw_d[:])    # dv
            nc.sync.dma_start(out=wb[:], in_=basew_d[:])   # base
            nc.vector.tensor_tensor(out=wc[:], in0=wa[:], in1=wb[:], op=A.add)
            nc.vector.tensor_scalar(out=wa[:], in0=wc[:], scalar1=0.0,
                                    scalar2=58.0, op0=A.max, op1=A.min)  # tcl
            nc.vector.tensor_copy(out=wi[:], in_=wa[:])    # round-to-nearest
            nc.vector.tensor_copy(out=wb[:], in_=wi[:])    # rf
            nc.vector.tensor_tensor(out=wc[:], in0=wb[:], in1=wa[:],
                                    op=A.is_gt)            # rf > tcl
            nc.vector.tensor_tensor(out=wa[:], in0=wb[:], in1=wc[:],
                                    op=A.subtract)         # floor
            qfw = sp.tile([16, K * SLOTS], F32, tag="qfw")
            nc.vector.scalar_tensor_tensor(
                out=qfw[:], in0=wa[:, :K * SLOTS], scalar=float(WP),
                in1=wa[:, K * SLOTS:], op0=A.mult, op1=A.add)
            nc.vector.tensor_copy(out=idxw[:16, :], in_=qfw[:])
            for r in (16, 32, 64):
                nc.sync.dma_start(out=idxw[r:2 * r, :], in_=idxw[0:r, :])

            # ---- Phase 2: padded planes in bf16, on GpSimd ----
            xp = sp.tile([128, Q], F32, tag="xp")
            nc.gpsimd.memset(xp[:], 0.0)
            xpv = xp[:].rearrange("c (h w) -> c h w", h=HP)
            nc.sync.dma_start(out=xpv[:, PD:PD + H, PD:PD + W],
                              in_=x_d[:].rearrange("c (h w) -> c h w", h=H))
            dxf = sp.tile([128, Q], F32, tag="dxf")    # x[q+1]-x[q] in f32
            nc.gpsimd.memset(dxf[:, Q - 1:], 0.0)
            nc.gpsimd.tensor_tensor(out=dxf[:, :Q - 1], in0=xp[:, 1:Q],
                                    in1=xp[:, :Q - 1], op=A.subtract)
            xpb = sp.tile([128, Q], BF16, tag="xpb")
            nc.gpsimd.tensor_copy(out=xpb[:], in_=xp[:])
            dyb = sp.tile([128, Q], BF16, tag="dyb")
            nc.gpsimd.memset(dyb[:, Q - WP:], 0.0)
            nc.gpsimd.tensor_tensor(out=dyb[:, :Q - WP], in0=xp[:, WP:Q],
                                    in1=xp[:, :Q - WP], op=A.subtract)
            dxb = sp.tile([128, Q], BF16, tag="dxb")
            nc.gpsimd.tensor_copy(out=dxb[:], in_=dxf[:])
            dxyb = sp.tile([128, Q], BF16, tag="dxyb")
            nc.gpsimd.memset(dxyb[:, Q - WP - 1:], 0.0)
            nc.gpsimd.tensor_tensor(out=dxyb[:, :Q - WP - 1],
                                    in0=dxf[:, WP:Q - 1],
                                    in1=dxf[:, :Q - WP - 1], op=A.subtract)
            planes = [xpb, dyb, dxb, dxyb]
            # two 128-blocks per PSUM tile/eviction: (0,1),(2,3),...,(28,)
            for pair in range(15):
                b0 = 2 * pair
                nb = 1 if b0 == 28 else 2
                pt = pp.tile([128, 2 * ELEM], BF16)
                for bi in range(nb):
                    b = b0 + bi
                    n = min(128, Q - b * 128)
                    for t, pl in enumerate(planes):
                        nc.tensor.transpose(
                            out=pt[:n, bi * ELEM + 128 * t:
                                   bi * ELEM + 128 * (t + 1)],
                            in_=pl[:, b * 128:b * 128 + n],
                            identity=idnb[:])
                ev = evp.tile([128, 2 * ELEM], BF16)
                rows = min(256, Q - b0 * 128)
                if nb == 2 and rows < 256:
                    # ragged pair: evict per block to keep row mapping simple
                    n0 = 128
                    n1 = rows - 128
                    nc.scalar.copy(out=ev[:n0, :ELEM], in_=pt[:n0, :ELEM])
                    nc.scalar.copy(out=ev[:n1, ELEM:], in_=pt[:n1, ELEM:])
                    nc.sync.dma_start(out=g4r[b0 * 128:b0 * 128 + n0, :],
                                      in_=ev[:n0, :ELEM])
                    nc.sync.dma_start(out=g4r[(b0 + 1) * 128:
                                              (b0 + 1) * 128 + n1, :],
                                      in_=ev[:n1, ELEM:])
                else:
                    nc.scalar.copy(out=ev[:min(rows, 128), :nb * ELEM],
                                   in_=pt[:min(rows, 128), :nb * ELEM])
                    nc.sync.dma_start(
                        out=g4r[b0 * 128:b0 * 128 + rows, :].rearrange(
                            "(b p) e -> p b e", b=nb),
                        in_=ev[:min(rows, 128), :nb * ELEM])

            # ---- Phase 1b: bilinear weights, packed layout, on DVE ----
            dv = sp.tile([126, FREE1], F32, tag="dv")
            nc.sync.dma_start(out=dv[:], in_=offd_d[:])
            bs = sp.tile([126, FREE1], F32, tag="bs")
            nc.sync.dma_start(out=bs[:], in_=based_d[:])
            tr = sp.tile([126, FREE1], F32, tag="tr")
            nc.vector.tensor_tensor(out=tr[:], in0=dv[:], in1=bs[:], op=A.add)
            tcl = sp.tile([126, FREE1], F32, tag="tcl")
            nc.vector.tensor_scalar(out=tcl[:], in0=tr[:], scalar1=0.0,
                                    scalar2=58.0, op0=A.max, op1=A.min)
            ri = sp.tile([126, FREE1], I32, tag="ri")
            nc.vector.tensor_copy(out=ri[:], in_=tcl[:])
            rf = sp.tile([126, FREE1], F32, tag="rf")
            nc.vector.tensor_copy(out=rf[:], in_=ri[:])
            gtt = sp.tile([126, FREE1], F32, tag="gtt")
            nc.vector.tensor_tensor(out=gtt[:], in0=rf[:], in1=tcl[:],
                                    op=A.is_gt)
            fl = sp.tile([126, FREE1], F32, tag="fl")
            nc.vector.tensor_tensor(out=fl[:], in0=rf[:], in1=gtt[:],
                                    op=A.subtract)
            wv = sp.tile([126, FREE1], F32, tag="wv")    # wy | wx
            nc.vector.tensor_tensor(out=wv[:], in0=tr[:], in1=fl[:],
                                    op=A.subtract)
            # reshuffle [126, 448] (a,k,s)xf -> [18, 3136] (a,k)x(s,f)
            # via DRAM (cross partition/free regrouping needs a flat hop)
            nc.sync.dma_start(out=wd[:], in_=wv[:])
            wsb2 = sp.tile([18, NPOS], F32, tag="wsb2")
            nc.sync.dma_start(
                out=wsb2[:],
                in_=wd[:].rearrange("(c s) f -> c (s f)", s=SPT))
            for b in range(NBLK):
                n = min(128, NPOS - b * 128)
                if n <= 0:
                    break
                ptw = pp2.tile([128, 32], F32)
                nc.tensor.transpose(out=ptw[:n, 0:18],
                                    in_=wsb2[:, b * 128:b * 128 + n],
                                    identity=idn[:18, :18])
                nc.scalar.copy(out=wsc[:n, b, :], in_=ptw[:n, 0:18])

        # ---------------- Phase 3: gather / combine / matmul ----------------
        with tc.tile_pool(name="gk", bufs=2) as gp, \
             tc.tile_pool(name="cp", bufs=2) as cpp, \
             tc.tile_pool(name="cols", bufs=2) as csp, \
             tc.tile_pool(name="uv", bufs=4) as uvp, \
             tc.tile_pool(name="accp", bufs=1, space="PSUM") as accp, \
             tc.tile_pool(name="tps", bufs=1, space="PSUM") as tpp:
            acc = accp.tile([128, NPOS], F32)
            for k in range(K):
                gk = gp.tile([128, NBLK, ELEM], BF16)
                for hb, nb in GSPLIT:
                    nc.gpsimd.dma_gather(
                        gk[:, hb:hb + nb, :], g4r[:],
                        idxw[:, k * SLOTS + hb * 8:k * SLOTS + (hb + nb) * 8],
                        num_idxs=nb * 128, num_idxs_reg=nb * 128,
                        elem_size=ELEM)
                colsP = cpp.tile([128, NPB], BF16)     # pos-major combined
                for b in range(NBLK):
                    wys = wsc[:, b, k:k + 1]
                    wxs = wsc[:, b, 9 + k:10 + k]
                    uv = uvp.tile([128, 256], BF16, tag="uv")
                    # uv = [x|Dy] + wx*[Dx|Dxy]  ->  [v', u']
                    nc.vector.scalar_tensor_tensor(
                        uv[:], gk[:, b, 256:512], wxs, gk[:, b, 0:256],
                        op0=A.mult, op1=A.add)
                    # cols = v' + wy*u'
                    nc.vector.scalar_tensor_tensor(
                        colsP[:, b * 128:(b + 1) * 128], uv[:, 128:256], wys,
                        uv[:, 0:128], op0=A.mult, op1=A.add)
                cols = csp.tile([128, NPB], BF16)      # c-major
                for g in range(7):
                    bs_ = list(range(4 * g, min(4 * g + 4, NBLK)))
                    ptc = tpp.tile([128, 512], BF16)
                    for j, b in enumerate(bs_):
                        nc.tensor.transpose(out=ptc[:, 128 * j:128 * (j + 1)],
                                            in_=colsP[:, b * 128:(b + 1) * 128],
                                            identity=idnb[:])
                    wdt = len(bs_) * 128
                    nc.scalar.copy(out=cols[:, 512 * g:512 * g + wdt],
                                   in_=ptc[:, :wdt])
                for ch in range(7):
                    lo = 512 * ch
                    hi = min(lo + 512, NPOS)
                    nc.tensor.matmul(acc[:, lo:hi],
                                     lhsT=w_sb[:, k * 128:(k + 1) * 128],
                                     rhs=cols[:, lo:hi],
                                     start=(k == 0), stop=(k == K - 1))

            # ---------------- epilogue ----------------
            tmp = csp.tile([128, NPOS], F32, tag="epi1")
            nc.vector.tensor_tensor(out=tmp[:], in0=acc[:], in1=x2_sb[:], op=A.add)
            outp = cpp.tile([128, NPOS], F32, tag="epi2")
            nc.scalar.activation(outp[:], tmp[:], mybir.ActivationFunctionType.Relu,
                                 bias=bias_sb[:], scale=1.0)
            nc.sync.dma_start(out=out_d[:], in_=outp[:])


def make_core_inputs(x, offset, weight, bias, x2):
    """Full inputs -> list of 8 per-core input dicts (host batch sharding)."""
    based, basew = host_consts()
    wt = np.ascontiguousarray(
        weight.reshape(128, 128, K).transpose(1, 2, 0).reshape(128, K * 128)
    ).astype(ml_dtypes.bfloat16)
    cores = []
    for i in range(N):
        off = offset[i].reshape(2 * K, NPOS).astype(np.float32)
        offd = np.ascontiguousarray(
            off.reshape(K, 2, SPT, FREE1).transpose(1, 0, 2, 3)
            .reshape(2 * K * SPT, FREE1))
        cores.append({
            "x": np.ascontiguousarray(x[i].reshape(C, NPOS), dtype=np.float32),
            "offd": offd,
            "offw": wrap_offsets(off),
            "x2": np.ascontiguousarray(x2[i].reshape(C, NPOS), dtype=np.float32),
            "wt": wt,
            "bias": np.ascontiguousarray(bias.reshape(C, 1), dtype=np.float32),
            "based": based,
            "basew": basew,
        })
    return cores


_CACHED_NC = None

IN_SPECS = [("x", (C, NPOS), F32), ("offd", (2 * K * SPT, FREE1), F32),
            ("offw", (16, 2 * K * SLOTS), F32), ("x2", (C, NPOS), F32),
            ("wt", (C, K * 128), BF16), ("bias", (C, 1), F32),
            ("based", (2 * K * SPT, FREE1), F32),
            ("basew", (16, 2 * K * SLOTS), F32)]


def _build_nc():
    global _CACHED_NC
    if _CACHED_NC is not None:
        return _CACHED_NC
    nc = bacc.Bacc("TRN2", target_bir_lowering=False, debug=False, num_devices=N)
    ins = [nc.dram_tensor(nm, list(sh), dt, kind="ExternalInput").ap()
           for nm, sh, dt in IN_SPECS]
    out = nc.dram_tensor("out", [C, NPOS], F32, kind="ExternalOutput").ap()
    with tile.TileContext(nc, trace_sim=False) as tc:
        build_kernel(tc, out, ins)
    nc.compile()
    _CACHED_NC = nc
    return nc


def run_cores(inputs, trace=False):
    """Run the SPMD kernel; returns (out [N,C,H,W] f32, exec_time_ns or None)."""
    nc = _build_nc()
    in_maps = make_core_inputs(inputs["x"], inputs["offset"], inputs["weight"],
                               inputs["bias"], inputs["x2"])
    res = bass_utils.run_bass_kernel_spmd(nc, in_maps, core_ids=list(range(N)),
                                          trace=trace)
    out = np.stack([res.results[i]["out"] for i in range(N)])
    return out.reshape(N, C, H, W), res.exec_time_ns


def kernel(x, offset, weight, bias, x2):
    x = np.asarray(x, dtype=np.float32)
    offset = np.asarray(offset, dtype=np.float32)
    weight = np.asarray(weight, dtype=np.float32)
    bias = np.asarray(bias, dtype=np.float32)
    x2 = np.asarray(x2, dtype=np.float32)
    out, _ = run_cores({"x": x, "offset": offset, "weight": weight,
                        "bias": bias, "x2": x2}, trace=False)
    return out


# revision 24
# speedup vs baseline: 1.5848x; 1.1145x over previous
"""Deformable conv2d + residual add + ReLU on 8 Trainium2 NeuronCores.

Self-contained harness entry: kernel(**inputs) -> np.ndarray.
Sharding: data-parallel over batch N=8 (one image per core); weight/bias
replicated. Each core runs the same Bass/Tile program.

v3 design (gather-descgen-bound pipeline, bf16 data path):
  Prologue (phases overlap across engines):
  A) gather-index chain on DVE directly in the SWDGE wrapped layout
     [16, (axis,k,slot)] (offsets pre-wrapped on host) -> no transposes or
     scatter DMAs; floor via x - mod(x,1). A second small chain in packed
     [126, 448] layout produces the bilinear weights, PE-transposed per
     128-position block into per-partition scalars.
  B) zero-padded image planes [x, Dy, Dx, Dxy] in bf16 built on the GpSimd
     engine (parallel with A on DVE), PE-transposed to q-major 1KB rows in
     DRAM (two 128-blocks per PSUM eviction).
  Main loop, per kernel-tap k:
  C) SWDGE dma_gather of 3200 sample rows (position-major, 2 calls),
     bilinear combine with 2 fused scalar_tensor_tensor ops per block
     ([x|Dy] + wx*[Dx|Dxy], then + wy*hi), PE transpose back to
     channel-major, bf16 matmul accumulated in PSUM.
  D) epilogue: + x2 + bias, ReLU, store f32.

Math: bilinear(x, py, px) = x[q] + wx*Dx[q] + wy*Dy[q] + wx*wy*Dxy[q] with
q = floor(py+PD)*WP + floor(px+PD) on the zero-padded grid; the zero
padding reproduces torchvision's out-of-bounds zeroing exactly, and clamping
floor() into the pad ring keeps fully-out-of-range samples at zero.
"""

import sys

for _p in ("/opt/trn_rl_repo",):
    if _p not in sys.path:
        sys.path.insert(0, _p)

import numpy as np
import ml_dtypes

import concourse.bacc as bacc
import concourse.mybir as mybir
import concourse.tile as tile
from concourse import bass_utils
from concourse.masks import make_identity

F32 = mybir.dt.float32
BF16 = mybir.dt.bfloat16
I32 = mybir.dt.int32
I16 = mybir.dt.int16
A = mybir.AluOpType

# problem constants (nn_DeformConvAddReLU2d: N=8, C=Cout=128, 56x56, 3x3)
N, C, H, W = 8, 128, 56, 56
K = 9
PD = 2
HP, WP = H + 2 * PD, W + 2 * PD          # 60, 60
Q = HP * WP                               # 3600
NPOS = H * W                              # 3136
NPB = 3200                                # samples per tap padded to 25 blocks
NBLK = NPB // 128                         # 25
ELEM = 512                                # row: [x|Dy|Dx|Dxy] x 128c bf16 (1KB)
SLOTS = NPB // 16                         # 200 wrapped idx slots per tap
SPT = 7                                   # 3136 = 7 * 448 partition packing
FREE1 = NPOS // SPT                       # 448
GSPLIT = [(0, 8), (8, 8), (16, 8), (24, 1)]   # gather call split (start, nblocks)


def host_consts():
    """Base sampling positions, pre-biased by +PD (padded-grid coords).

    Returns:
      based: [126, 448] f32 — deinterleaved packed layout (axis, k, s) x f
             for the weight chain.
      basew: [16, 2*K*SLOTS] f32 — SWDGE-wrapped layout p x (axis, k, slot)
             for the gather-index chain; padded tail positions get -1000 so
             they clamp to q=0 (a guaranteed-zero pad row).
    """
    ki = np.arange(3).repeat(3)
    kj = np.tile(np.arange(3), 3)
    i = np.arange(H)
    j = np.arange(W)
    by = (i[None, :, None] + ki[:, None, None] + 1).astype(np.float32)
    bx = (j[None, None, :] + kj[:, None, None] + 1).astype(np.float32)
    by = np.broadcast_to(by, (K, H, W)).reshape(K, NPOS)
    bx = np.broadcast_to(bx, (K, H, W)).reshape(K, NPOS)
    based = np.concatenate(
        [by.reshape(K * SPT, FREE1), bx.reshape(K * SPT, FREE1)], axis=0
    ).astype(np.float32)

    byp = np.full((K, NPB), -1000.0, dtype=np.float32)
    bxp = np.full((K, NPB), -1000.0, dtype=np.float32)
    byp[:, :NPOS] = by
    bxp[:, :NPOS] = bx
    # wrap: [K, SLOTS, 16] -> [16, K, SLOTS]
    byw = byp.reshape(K, SLOTS, 16).transpose(2, 0, 1).reshape(16, K * SLOTS)
    bxw = bxp.reshape(K, SLOTS, 16).transpose(2, 0, 1).reshape(16, K * SLOTS)
    basew = np.concatenate([byw, bxw], axis=1)
    return based, np.ascontiguousarray(basew)


def wrap_offsets(off):
    """off [2K, NPOS] f32 -> SWDGE-wrapped [16, 2*K*SLOTS] (axis, k, slot)."""
    offp = np.zeros((2 * K, NPB), dtype=np.float32)
    offp[:, :NPOS] = off
    w = offp.reshape(K, 2, SLOTS, 16).transpose(3, 1, 0, 2)  # [16, 2, K, SLOTS]
    return np.ascontiguousarray(w.reshape(16, 2 * K * SLOTS))


def build_kernel(tc, outs, ins):
    nc = tc.nc
    out_d = outs                                   # [128, NPOS] f32
    x_d, offd_d, offw_d, x2_d, wt_d, bias_d, based_d, basew_d = ins

    with tc.tile_pool(name="persist", bufs=1) as pers, \
         tc.tile_pool(name="dram", bufs=1, space="DRAM") as dp:
        g4r = dp.tile([Q, ELEM], BF16)
        wd = dp.tile([126, FREE1], F32)

        idn = pers.tile([128, 128], F32)
        make_identity(nc, idn[:])
        idnb = pers.tile([128, 128], BF16)
        nc.vector.tensor_copy(out=idnb[:], in_=idn[:])
        wsc = pers.tile([128, NBLK, 18], BF16)     # scalars: wy at k, wx at 9+k
        nc.vector.memset(wsc[:], 0.0)
        idxw = pers.tile([128, K * SLOTS], I16)    # wrapped gather indices
        w_sb = pers.tile([128, K * 128], BF16)     # lhsT per tap: [c, o]
        bias_sb = pers.tile([128, 1], F32)
        x2b = pers.tile([128, NPOS], BF16)

        nc.sync.dma_start(out=w_sb[:], in_=wt_d[:])
        nc.sync.dma_start(out=bias_sb[:], in_=bias_d[:])
        nc.sync.dma_start(out=x2b[:], in_=x2_d[:])

        # =============== Prologue: phases 1a/2/1b overlap ==================
        with tc.tile_pool(name="prosb", bufs=1) as sp, \
             tc.tile_pool(name="proev", bufs=3) as evp, \
             tc.tile_pool(name="props", bufs=2, space="PSUM") as pp, \
             tc.tile_pool(name="props2", bufs=2, space="PSUM") as pp2:
            # ---- Phase 1a: gather indices, wrapped layout, on DVE ----
            # 4 rotating [16, M] buffers (wa/wb/wc f32, wi i32) keep SBUF flat
            M = 2 * K * SLOTS                      # 3600
            wa = sp.tile([16, M], F32, tag="wa")
            wb = sp.tile([16, M], F32, tag="wb")
            wc = sp.tile([16, M], F32, tag="wc")
            wi = sp.tile([16, M], I32, tag="wi")
            nc.sync.dma_start(out=wa[:], in_=offw_d[:])    # dv
            nc.sync.dma_start(out=wb[:], in_=basew_d[:])   # base
            nc.vector.tensor_tensor(out=wc[:], in0=wa[:], in1=wb[:], op=A.add)
            nc.vector.tensor_scalar(out=wa[:], in0=wc[:], scalar1=0.0,
                                    scalar2=58.0, op0=A.max, op1=A.min)  # tcl
            nc.vector.tensor_copy(out=wi[:], in_=wa[:])    # round-to-nearest
            nc.vector.tensor_copy(out=wb[:], in_=wi[:])    # rf
            nc.vector.tensor_tensor(out=wc[:], in0=wb[:], in1=wa[:],
                                    op=A.is_gt)            # rf > tcl
            nc.vector.tensor_tensor(out=wa[:], in0=wb[:], in1=wc[:],
                                    op=A.subtract)         # floor
            qfw = sp.tile([16, K * SLOTS], F32, tag="qfw")
            nc.vector.scalar_tensor_tensor(
                out=qfw[:], in0=wa[:, :K * SLOTS], scalar=float(WP),
                in1=wa[:, K * SLOTS:], op0=A.mult, op1=A.add)
            nc.vector.tensor_copy(out=idxw[:16, :], in_=qfw[:])
            for r in (16, 32, 64):
                nc.sync.dma_start(out=idxw[r:2 * r, :], in_=idxw[0:r, :])

            # ---- Phase 2: padded planes in bf16, on DVE ----
            # x loads contiguously (1 packet/partition); DVE inserts padding
            # via a strided-write copy (the strided DMA was ~7K packets).
            xf = sp.tile([128, NPOS], F32, tag="xf")
            nc.sync.dma_start(out=xf[:], in_=x_d[:])
            xp = sp.tile([128, Q], F32, tag="xp")
            nc.vector.memset(xp[:], 0.0)
            xpv = xp[:].rearrange("c (h w) -> c h w", h=HP)
            nc.vector.tensor_copy(
                out=xpv[:, PD:PD + H, PD:PD + W],
                in_=xf[:].rearrange("c (h w) -> c h w", h=H))
            dxf = sp.tile([128, Q], F32, tag="dxf")    # x[q+1]-x[q] in f32
            nc.vector.memset(dxf[:, Q - 1:], 0.0)
            nc.vector.tensor_tensor(out=dxf[:, :Q - 1], in0=xp[:, 1:Q],
                                    in1=xp[:, :Q - 1], op=A.subtract)
            xpb = sp.tile([128, Q], BF16, tag="xpb")
            nc.vector.tensor_copy(out=xpb[:], in_=xp[:])
            dyb = sp.tile([128, Q], BF16, tag="dyb")
            nc.vector.memset(dyb[:, Q - WP:], 0.0)
            nc.vector.tensor_tensor(out=dyb[:, :Q - WP], in0=xp[:, WP:Q],
                                    in1=xp[:, :Q - WP], op=A.subtract)
            dxb = sp.tile([128, Q], BF16, tag="dxb")
            nc.vector.tensor_copy(out=dxb[:], in_=dxf[:])
            dxyb = sp.tile([128, Q], BF16, tag="dxyb")
            nc.vector.memset(dxyb[:, Q - WP - 1:], 0.0)
            nc.vector.tensor_tensor(out=dxyb[:, :Q - WP - 1],
                                    in0=dxf[:, WP:Q - 1],
                                    in1=dxf[:, :Q - WP - 1], op=A.subtract)
            planes = [xpb, dyb, dxb, dxyb]
            # two 128-blocks per PSUM tile/eviction: (0,1),(2,3),...,(28,)
            for pair in range(15):
                b0 = 2 * pair
                nb = 1 if b0 == 28 else 2
                pt = pp.tile([128, 2 * ELEM], BF16)
                for bi in range(nb):
                    b = b0 + bi
                    n = min(128, Q - b * 128)
                    for t, pl in enumerate(planes):
                        nc.tensor.transpose(
                            out=pt[:n, bi * ELEM + 128 * t:
                                   bi * ELEM + 128 * (t + 1)],
                            in_=pl[:, b * 128:b * 128 + n],
                            identity=idnb[:])
                ev = evp.tile([128, 2 * ELEM], BF16)
                rows = min(256, Q - b0 * 128)
                if nb == 2 and rows < 256:
                    # ragged pair: evict per block to keep row mapping simple
                    n0 = 128
                    n1 = rows - 128
                    nc.scalar.copy(out=ev[:n0, :ELEM], in_=pt[:n0, :ELEM])
                    nc.scalar.copy(out=ev[:n1, ELEM:], in_=pt[:n1, ELEM:])
                    nc.sync.dma_start(out=g4r[b0 * 128:b0 * 128 + n0, :],
                                      in_=ev[:n0, :ELEM])
                    nc.sync.dma_start(out=g4r[(b0 + 1) * 128:
                                              (b0 + 1) * 128 + n1, :],
                                      in_=ev[:n1, ELEM:])
                else:
                    nc.scalar.copy(out=ev[:min(rows, 128), :nb * ELEM],
                                   in_=pt[:min(rows, 128), :nb * ELEM])
                    nc.sync.dma_start(
                        out=g4r[b0 * 128:b0 * 128 + rows, :].rearrange(
                            "(b p) e -> p b e", b=nb),
                        in_=ev[:min(rows, 128), :nb * ELEM])

            # ---- Phase 1b: bilinear weights, packed layout, on DVE ----
            dv = sp.tile([126, FREE1], F32, tag="dv")
            nc.sync.dma_start(out=dv[:], in_=offd_d[:])
            bs = sp.tile([126, FREE1], F32, tag="bs")
            nc.sync.dma_start(out=bs[:], in_=based_d[:])
            tr = sp.tile([126, FREE1], F32, tag="tr")
            nc.vector.tensor_tensor(out=tr[:], in0=dv[:], in1=bs[:], op=A.add)
            tcl = sp.tile([126, FREE1], F32, tag="tcl")
            nc.vector.tensor_scalar(out=tcl[:], in0=tr[:], scalar1=0.0,
                                    scalar2=58.0, op0=A.max, op1=A.min)
            ri = sp.tile([126, FREE1], I32, tag="ri")
            nc.vector.tensor_copy(out=ri[:], in_=tcl[:])
            rf = sp.tile([126, FREE1], F32, tag="rf")
            nc.vector.tensor_copy(out=rf[:], in_=ri[:])
            gtt = sp.tile([126, FREE1], F32, tag="gtt")
            nc.vector.tensor_tensor(out=gtt[:], in0=rf[:], in1=tcl[:],
                                    op=A.is_gt)
            fl = sp.tile([126, FREE1], F32, tag="fl")
            nc.vector.tensor_tensor(out=fl[:], in0=rf[:], in1=gtt[:],
                                    op=A.subtract)
            wv = sp.tile([126, FREE1], F32, tag="wv")    # wy | wx
            nc.vector.tensor_tensor(out=wv[:], in0=tr[:], in1=fl[:],
                                    op=A.subtract)
            # reshuffle [126, 448] (a,k,s)xf -> [18, 3136] (a,k)x(s,f)
            # via DRAM (cross partition/free regrouping needs a flat hop)
            nc.sync.dma_start(out=wd[:], in_=wv[:])
            wsb2 = sp.tile([18, NPOS], F32, tag="wsb2")
            nc.sync.dma_start(
                out=wsb2[:],
                in_=wd[:].rearrange("(c s) f -> c (s f)", s=SPT))
            for b in range(NBLK):
                n = min(128, NPOS - b * 128)
                if n <= 0:
                    break
                ptw = pp2.tile([128, 32], F32)
                nc.tensor.transpose(out=ptw[:n, 0:18],
                                    in_=wsb2[:, b * 128:b * 128 + n],
                                    identity=idn[:18, :18])
                nc.scalar.copy(out=wsc[:n, b, :], in_=ptw[:n, 0:18])

        # ---------------- Phase 3: gather / combine / matmul ----------------
        with tc.tile_pool(name="gk", bufs=3) as gp, \
             tc.tile_pool(name="cp", bufs=2) as cpp, \
             tc.tile_pool(name="cols", bufs=2) as csp, \
             tc.tile_pool(name="uv", bufs=4) as uvp, \
             tc.tile_pool(name="accp", bufs=1, space="PSUM") as accp, \
             tc.tile_pool(name="tps", bufs=1, space="PSUM") as tpp:
            acc = accp.tile([128, NPOS], F32)
            for k in range(K):
                gk = gp.tile([128, NBLK, ELEM], BF16)
                for hb, nb in GSPLIT:
                    nc.gpsimd.dma_gather(
                        gk[:, hb:hb + nb, :], g4r[:],
                        idxw[:, k * SLOTS + hb * 8:k * SLOTS + (hb + nb) * 8],
                        num_idxs=nb * 128, num_idxs_reg=nb * 128,
                        elem_size=ELEM)
                colsP = cpp.tile([128, NPB], BF16)     # pos-major combined
                for b in range(NBLK):
                    wys = wsc[:, b, k:k + 1]
                    wxs = wsc[:, b, 9 + k:10 + k]
                    uv = uvp.tile([128, 256], BF16, tag="uv")
                    # uv = [x|Dy] + wx*[Dx|Dxy]  ->  [v', u']
                    nc.vector.scalar_tensor_tensor(
                        uv[:], gk[:, b, 256:512], wxs, gk[:, b, 0:256],
                        op0=A.mult, op1=A.add)
                    # cols = v' + wy*u'
                    nc.vector.scalar_tensor_tensor(
                        colsP[:, b * 128:(b + 1) * 128], uv[:, 128:256], wys,
                        uv[:, 0:128], op0=A.mult, op1=A.add)
                cols = csp.tile([128, NPB], BF16)      # c-major
                for g in range(7):
                    bs_ = list(range(4 * g, min(4 * g + 4, NBLK)))
                    ptc = tpp.tile([128, 512], BF16)
                    for j, b in enumerate(bs_):
                        nc.tensor.transpose(out=ptc[:, 128 * j:128 * (j + 1)],
                                            in_=colsP[:, b * 128:(b + 1) * 128],
                                            identity=idnb[:])
                    wdt = len(bs_) * 128
                    nc.scalar.copy(out=cols[:, 512 * g:512 * g + wdt],
                                   in_=ptc[:, :wdt])
                for ch in range(7):
                    lo = 512 * ch
                    hi = min(lo + 512, NPOS)
                    nc.tensor.matmul(acc[:, lo:hi],
                                     lhsT=w_sb[:, k * 128:(k + 1) * 128],
                                     rhs=cols[:, lo:hi],
                                     start=(k == 0), stop=False)

            # ------- epilogue: += x2 on PE, then chunked ReLU + store -------
            outp = cpp.tile([128, NPOS], F32, tag="epi2")
            for ch in range(7):
                lo = 512 * ch
                hi = min(lo + 512, NPOS)
                nc.tensor.matmul(acc[:, lo:hi], lhsT=idnb[:],
                                 rhs=x2b[:, lo:hi], start=False, stop=True)
                nc.scalar.activation(outp[:, lo:hi], acc[:, lo:hi],
                                     mybir.ActivationFunctionType.Relu,
                                     bias=bias_sb[:], scale=1.0)
                nc.sync.dma_start(out=out_d[:, lo:hi], in_=outp[:, lo:hi])


def make_core_inputs(x, offset, weight, bias, x2):
    """Full inputs -> list of 8 per-core input dicts (host batch sharding)."""
    based, basew = host_consts()
    wt = np.ascontiguousarray(
        weight.reshape(128, 128, K).transpose(1, 2, 0).reshape(128, K * 128)
    ).astype(ml_dtypes.bfloat16)
    cores = []
    for i in range(N):
        off = offset[i].reshape(2 * K, NPOS).astype(np.float32)
        offd = np.ascontiguousarray(
            off.reshape(K, 2, SPT, FREE1).transpose(1, 0, 2, 3)
            .reshape(2 * K * SPT, FREE1))
        cores.append({
            "x": np.ascontiguousarray(x[i].reshape(C, NPOS), dtype=np.float32),
            "offd": offd,
            "offw": wrap_offsets(off),
            "x2": np.ascontiguousarray(
                x2[i].reshape(C, NPOS)).astype(ml_dtypes.bfloat16),
            "wt": wt,
            "bias": np.ascontiguousarray(bias.reshape(C, 1), dtype=np.float32),
            "based": based,
            "basew": basew,
        })
    return cores


_CACHED_NC = None

IN_SPECS = [("x", (C, NPOS), F32), ("offd", (2 * K * SPT, FREE1), F32),
            ("offw", (16, 2 * K * SLOTS), F32), ("x2", (C, NPOS), BF16),
            ("wt", (C, K * 128), BF16), ("bias", (C, 1), F32),
            ("based", (2 * K * SPT, FREE1), F32),
            ("basew", (16, 2 * K * SLOTS), F32)]


def _build_nc():
    global _CACHED_NC
    if _CACHED_NC is not None:
        return _CACHED_NC
    nc = bacc.Bacc("TRN2", target_bir_lowering=False, debug=False, num_devices=N)
    ins = [nc.dram_tensor(nm, list(sh), dt, kind="ExternalInput").ap()
           for nm, sh, dt in IN_SPECS]
    out = nc.dram_tensor("out", [C, NPOS], F32, kind="ExternalOutput").ap()
    with tile.TileContext(nc, trace_sim=False) as tc:
        build_kernel(tc, out, ins)
    nc.compile()
    _CACHED_NC = nc
    return nc


def run_cores(inputs, trace=False):
    """Run the SPMD kernel; returns (out [N,C,H,W] f32, exec_time_ns or None)."""
    nc = _build_nc()
    in_maps = make_core_inputs(inputs["x"], inputs["offset"], inputs["weight"],
                               inputs["bias"], inputs["x2"])
    res = bass_utils.run_bass_kernel_spmd(nc, in_maps, core_ids=list(range(N)),
                                          trace=trace)
    out = np.stack([res.results[i]["out"] for i in range(N)])
    return out.reshape(N, C, H, W), res.exec_time_ns


def kernel(x, offset, weight, bias, x2):
    x = np.asarray(x, dtype=np.float32)
    offset = np.asarray(offset, dtype=np.float32)
    weight = np.asarray(weight, dtype=np.float32)
    bias = np.asarray(bias, dtype=np.float32)
    x2 = np.asarray(x2, dtype=np.float32)
    out, _ = run_cores({"x": x, "offset": offset, "weight": weight,
                        "bias": bias, "x2": x2}, trace=False)
    return out


# revision 26
# speedup vs baseline: 1.6170x; 1.0203x over previous
"""Deformable conv2d + residual add + ReLU on 8 Trainium2 NeuronCores.

Self-contained harness entry: kernel(**inputs) -> np.ndarray.
Sharding: data-parallel over batch N=8 (one image per core); weight/bias
replicated. Each core runs the same Bass/Tile program.

v3 design (gather-descgen-bound pipeline, bf16 data path):
  Prologue (phases overlap across engines):
  A) gather-index chain on DVE directly in the SWDGE wrapped layout
     [16, (axis,k,slot)] (offsets pre-wrapped on host) -> no transposes or
     scatter DMAs; floor via x - mod(x,1). A second small chain in packed
     [126, 448] layout produces the bilinear weights, PE-transposed per
     128-position block into per-partition scalars.
  B) zero-padded image planes [x, Dy, Dx, Dxy] in bf16 built on the GpSimd
     engine (parallel with A on DVE), PE-transposed to q-major 1KB rows in
     DRAM (two 128-blocks per PSUM eviction).
  Main loop, per kernel-tap k:
  C) SWDGE dma_gather of 3200 sample rows (position-major, 2 calls),
     bilinear combine with 2 fused scalar_tensor_tensor ops per block
     ([x|Dy] + wx*[Dx|Dxy], then + wy*hi), PE transpose back to
     channel-major, bf16 matmul accumulated in PSUM.
  D) epilogue: + x2 + bias, ReLU, store f32.

Math: bilinear(x, py, px) = x[q] + wx*Dx[q] + wy*Dy[q] + wx*wy*Dxy[q] with
q = floor(py+PD)*WP + floor(px+PD) on the zero-padded grid; the zero
padding reproduces torchvision's out-of-bounds zeroing exactly, and clamping
floor() into the pad ring keeps fully-out-of-range samples at zero.
"""

import sys

for _p in ("/opt/trn_rl_repo",):
    if _p not in sys.path:
        sys.path.insert(0, _p)

import numpy as np
import ml_dtypes

import concourse.bacc as bacc
import concourse.mybir as mybir
import concourse.tile as tile
from concourse import bass_utils
from concourse.masks import make_identity

F32 = mybir.dt.float32
BF16 = mybir.dt.bfloat16
I32 = mybir.dt.int32
I16 = mybir.dt.int16
A = mybir.AluOpType

# problem constants (nn_DeformConvAddReLU2d: N=8, C=Cout=128, 56x56, 3x3)
N, C, H, W = 8, 128, 56, 56
K = 9
PD = 2
HP, WP = H + 2 * PD, W + 2 * PD          # 60, 60
Q = HP * WP                               # 3600
NPOS = H * W                              # 3136
NPB = 3200                                # samples per tap padded to 25 blocks
NBLK = NPB // 128                         # 25
ELEM = 512                                # row: [x|Dy|Dx|Dxy] x 128c bf16 (1KB)
SLOTS = NPB // 16                         # 200 wrapped idx slots per tap
SPT = 7                                   # 3136 = 7 * 448 partition packing
FREE1 = NPOS // SPT                       # 448
GSPLIT = [(0, 8), (8, 8), (16, 8), (24, 1)]   # gather call split (start, nblocks)


def host_consts():
    """Base sampling positions, pre-biased by +PD (padded-grid coords).

    Returns:
      based: [126, 448] f32 — deinterleaved packed layout (axis, k, s) x f
             for the weight chain.
      basew: [16, 2*K*SLOTS] f32 — SWDGE-wrapped layout p x (axis, k, slot)
             for the gather-index chain; padded tail positions get -1000 so
             they clamp to q=0 (a guaranteed-zero pad row).
    """
    ki = np.arange(3).repeat(3)
    kj = np.tile(np.arange(3), 3)
    i = np.arange(H)
    j = np.arange(W)
    by = (i[None, :, None] + ki[:, None, None] + 1).astype(np.float32)
    bx = (j[None, None, :] + kj[:, None, None] + 1).astype(np.float32)
    by = np.broadcast_to(by, (K, H, W)).reshape(K, NPOS)
    bx = np.broadcast_to(bx, (K, H, W)).reshape(K, NPOS)
    based = np.concatenate(
        [by.reshape(K * SPT, FREE1), bx.reshape(K * SPT, FREE1)], axis=0
    ).astype(np.float32)

    byp = np.full((K, NPB), -1000.0, dtype=np.float32)
    bxp = np.full((K, NPB), -1000.0, dtype=np.float32)
    byp[:, :NPOS] = by
    bxp[:, :NPOS] = bx
    # wrap: [K, SLOTS, 16] -> [16, K, SLOTS]
    byw = byp.reshape(K, SLOTS, 16).transpose(2, 0, 1).reshape(16, K * SLOTS)
    bxw = bxp.reshape(K, SLOTS, 16).transpose(2, 0, 1).reshape(16, K * SLOTS)
    basew = np.concatenate([byw, bxw], axis=1)
    return based, np.ascontiguousarray(basew)


def wrap_offsets(off):
    """off [2K, NPOS] f32 -> SWDGE-wrapped [16, 2*K*SLOTS] (axis, k, slot)."""
    offp = np.zeros((2 * K, NPB), dtype=np.float32)
    offp[:, :NPOS] = off
    w = offp.reshape(K, 2, SLOTS, 16).transpose(3, 1, 0, 2)  # [16, 2, K, SLOTS]
    return np.ascontiguousarray(w.reshape(16, 2 * K * SLOTS))


def build_kernel(tc, outs, ins):
    nc = tc.nc
    out_d = outs                                   # [128, NPOS] f32
    x_d, offd_d, offw_d, x2_d, wt_d, bias_d, based_d, basew_d = ins

    with tc.tile_pool(name="persist", bufs=1) as pers, \
         tc.tile_pool(name="dram", bufs=1, space="DRAM") as dp:
        g4r = dp.tile([Q, ELEM], BF16)
        wd = dp.tile([126, FREE1], F32)

        idn = pers.tile([128, 128], F32)
        make_identity(nc, idn[:])
        idnb = pers.tile([128, 128], BF16)
        nc.vector.tensor_copy(out=idnb[:], in_=idn[:])
        wsc = pers.tile([128, NBLK, 18], BF16)     # scalars: wy at k, wx at 9+k
        nc.vector.memset(wsc[:], 0.0)
        idxw = pers.tile([128, K * SLOTS], I16)    # wrapped gather indices
        w_sb = pers.tile([128, K * 128], BF16)     # lhsT per tap: [c, o]
        bias_sb = pers.tile([128, 1], F32)
        x2b = pers.tile([128, NPOS], BF16)

        nc.sync.dma_start(out=w_sb[:], in_=wt_d[:])
        nc.sync.dma_start(out=bias_sb[:], in_=bias_d[:])
        nc.sync.dma_start(out=x2b[:], in_=x2_d[:])

        # =============== Prologue: phases 1a/2/1b overlap ==================
        with tc.tile_pool(name="prosb", bufs=1) as sp, \
             tc.tile_pool(name="proev", bufs=3) as evp, \
             tc.tile_pool(name="props", bufs=2, space="PSUM") as pp, \
             tc.tile_pool(name="props2", bufs=2, space="PSUM") as pp2:
            # ---- Phase 2: padded planes in bf16, on DVE ----
            # x loads contiguously (1 packet/partition); DVE inserts padding
            # via a strided-write copy (the strided DMA was ~7K packets).
            xf = sp.tile([128, NPOS], F32, tag="xf")
            nc.sync.dma_start(out=xf[:], in_=x_d[:])
            xp = sp.tile([128, Q], F32, tag="xp")
            nc.vector.memset(xp[:], 0.0)
            xpv = xp[:].rearrange("c (h w) -> c h w", h=HP)
            nc.vector.tensor_copy(
                out=xpv[:, PD:PD + H, PD:PD + W],
                in_=xf[:].rearrange("c (h w) -> c h w", h=H))
            dxf = sp.tile([128, Q], F32, tag="dxf")    # x[q+1]-x[q] in f32
            nc.vector.memset(dxf[:, Q - 1:], 0.0)
            nc.vector.tensor_tensor(out=dxf[:, :Q - 1], in0=xp[:, 1:Q],
                                    in1=xp[:, :Q - 1], op=A.subtract)
            xpb = sp.tile([128, Q], BF16, tag="xpb")
            nc.vector.tensor_copy(out=xpb[:], in_=xp[:])
            dyb = sp.tile([128, Q], BF16, tag="dyb")
            nc.vector.memset(dyb[:, Q - WP:], 0.0)
            nc.vector.tensor_tensor(out=dyb[:, :Q - WP], in0=xp[:, WP:Q],
                                    in1=xp[:, :Q - WP], op=A.subtract)
            dxb = sp.tile([128, Q], BF16, tag="dxb")
            nc.vector.tensor_copy(out=dxb[:], in_=dxf[:])
            dxyb = sp.tile([128, Q], BF16, tag="dxyb")
            nc.vector.memset(dxyb[:, Q - WP - 1:], 0.0)
            nc.vector.tensor_tensor(out=dxyb[:, :Q - WP - 1],
                                    in0=dxf[:, WP:Q - 1],
                                    in1=dxf[:, :Q - WP - 1], op=A.subtract)
            # ---- Phase 1a: gather indices, wrapped layout, on DVE ----
            # 4 rotating [16, M] buffers (wa/wb/wc f32, wi i32) keep SBUF flat
            M = 2 * K * SLOTS                      # 3600
            wa = sp.tile([16, M], F32, tag="wa")
            wb = sp.tile([16, M], F32, tag="wb")
            wc = sp.tile([16, M], F32, tag="wc")
            wi = sp.tile([16, M], I32, tag="wi")
            nc.sync.dma_start(out=wa[:], in_=offw_d[:])    # dv
            nc.sync.dma_start(out=wb[:], in_=basew_d[:])   # base
            nc.vector.tensor_tensor(out=wc[:], in0=wa[:], in1=wb[:], op=A.add)
            nc.vector.tensor_scalar(out=wa[:], in0=wc[:], scalar1=0.0,
                                    scalar2=58.0, op0=A.max, op1=A.min)  # tcl
            nc.vector.tensor_copy(out=wi[:], in_=wa[:])    # round-to-nearest
            nc.vector.tensor_copy(out=wb[:], in_=wi[:])    # rf
            nc.vector.tensor_tensor(out=wc[:], in0=wb[:], in1=wa[:],
                                    op=A.is_gt)            # rf > tcl
            nc.vector.tensor_tensor(out=wa[:], in0=wb[:], in1=wc[:],
                                    op=A.subtract)         # floor
            qfw = sp.tile([16, K * SLOTS], F32, tag="qfw")
            nc.vector.scalar_tensor_tensor(
                out=qfw[:], in0=wa[:, :K * SLOTS], scalar=float(WP),
                in1=wa[:, K * SLOTS:], op0=A.mult, op1=A.add)
            nc.vector.tensor_copy(out=idxw[:16, :], in_=qfw[:])
            for r in (16, 32, 64):
                nc.sync.dma_start(out=idxw[r:2 * r, :], in_=idxw[0:r, :])

            planes = [xpb, dyb, dxb, dxyb]
            # two 128-blocks per PSUM tile/eviction: (0,1),(2,3),...,(28,)
            for pair in range(15):
                b0 = 2 * pair
                nb = 1 if b0 == 28 else 2
                pt = pp.tile([128, 2 * ELEM], BF16)
                for bi in range(nb):
                    b = b0 + bi
                    n = min(128, Q - b * 128)
                    for t, pl in enumerate(planes):
                        nc.tensor.transpose(
                            out=pt[:n, bi * ELEM + 128 * t:
                                   bi * ELEM + 128 * (t + 1)],
                            in_=pl[:, b * 128:b * 128 + n],
                            identity=idnb[:])
                ev = evp.tile([128, 2 * ELEM], BF16)
                rows = min(256, Q - b0 * 128)
                if nb == 2 and rows < 256:
                    # ragged pair: evict per block to keep row mapping simple
                    n0 = 128
                    n1 = rows - 128
                    nc.scalar.copy(out=ev[:n0, :ELEM], in_=pt[:n0, :ELEM])
                    nc.scalar.copy(out=ev[:n1, ELEM:], in_=pt[:n1, ELEM:])
                    nc.sync.dma_start(out=g4r[b0 * 128:b0 * 128 + n0, :],
                                      in_=ev[:n0, :ELEM])
                    nc.sync.dma_start(out=g4r[(b0 + 1) * 128:
                                              (b0 + 1) * 128 + n1, :],
                                      in_=ev[:n1, ELEM:])
                else:
                    nc.scalar.copy(out=ev[:min(rows, 128), :nb * ELEM],
                                   in_=pt[:min(rows, 128), :nb * ELEM])
                    nc.sync.dma_start(
                        out=g4r[b0 * 128:b0 * 128 + rows, :].rearrange(
                            "(b p) e -> p b e", b=nb),
                        in_=ev[:min(rows, 128), :nb * ELEM])

            # ---- Phase 1b: bilinear weights, packed layout, on DVE ----
            dv = sp.tile([126, FREE1], F32, tag="dv")
            nc.sync.dma_start(out=dv[:], in_=offd_d[:])
            bs = sp.tile([126, FREE1], F32, tag="bs")
            nc.sync.dma_start(out=bs[:], in_=based_d[:])
            tr = sp.tile([126, FREE1], F32, tag="tr")
            nc.vector.tensor_tensor(out=tr[:], in0=dv[:], in1=bs[:], op=A.add)
            tcl = sp.tile([126, FREE1], F32, tag="tcl")
            nc.vector.tensor_scalar(out=tcl[:], in0=tr[:], scalar1=0.0,
                                    scalar2=58.0, op0=A.max, op1=A.min)
            ri = sp.tile([126, FREE1], I32, tag="ri")
            nc.vector.tensor_copy(out=ri[:], in_=tcl[:])
            rf = sp.tile([126, FREE1], F32, tag="rf")
            nc.vector.tensor_copy(out=rf[:], in_=ri[:])
            gtt = sp.tile([126, FREE1], F32, tag="gtt")
            nc.vector.tensor_tensor(out=gtt[:], in0=rf[:], in1=tcl[:],
                                    op=A.is_gt)
            fl = sp.tile([126, FREE1], F32, tag="fl")
            nc.vector.tensor_tensor(out=fl[:], in0=rf[:], in1=gtt[:],
                                    op=A.subtract)
            wv = sp.tile([126, FREE1], F32, tag="wv")    # wy | wx
            nc.vector.tensor_tensor(out=wv[:], in0=tr[:], in1=fl[:],
                                    op=A.subtract)
            # reshuffle [126, 448] (a,k,s)xf -> [18, 3136] (a,k)x(s,f)
            # via DRAM (cross partition/free regrouping needs a flat hop)
            nc.sync.dma_start(out=wd[:], in_=wv[:])
            wsb2 = sp.tile([18, NPOS], F32, tag="wsb2")
            nc.sync.dma_start(
                out=wsb2[:],
                in_=wd[:].rearrange("(c s) f -> c (s f)", s=SPT))
            for b in range(NBLK):
                n = min(128, NPOS - b * 128)
                if n <= 0:
                    break
                ptw = pp2.tile([128, 32], F32)
                nc.tensor.transpose(out=ptw[:n, 0:18],
                                    in_=wsb2[:, b * 128:b * 128 + n],
                                    identity=idn[:18, :18])
                nc.scalar.copy(out=wsc[:n, b, :], in_=ptw[:n, 0:18])

        # ---------------- Phase 3: gather / combine / matmul ----------------
        with tc.tile_pool(name="gk", bufs=3) as gp, \
             tc.tile_pool(name="cp", bufs=2) as cpp, \
             tc.tile_pool(name="cols", bufs=2) as csp, \
             tc.tile_pool(name="uv", bufs=4) as uvp, \
             tc.tile_pool(name="accp", bufs=1, space="PSUM") as accp, \
             tc.tile_pool(name="tps", bufs=1, space="PSUM") as tpp:
            acc = accp.tile([128, NPOS], F32)
            for k in range(K):
                gk = gp.tile([128, NBLK, ELEM], BF16)
                for hb, nb in GSPLIT:
                    nc.gpsimd.dma_gather(
                        gk[:, hb:hb + nb, :], g4r[:],
                        idxw[:, k * SLOTS + hb * 8:k * SLOTS + (hb + nb) * 8],
                        num_idxs=nb * 128, num_idxs_reg=nb * 128,
                        elem_size=ELEM)
                colsP = cpp.tile([128, NPB], BF16)     # pos-major combined
                for b in range(NBLK):
                    wys = wsc[:, b, k:k + 1]
                    wxs = wsc[:, b, 9 + k:10 + k]
                    uv = uvp.tile([128, 256], BF16, tag="uv")
                    # uv = [x|Dy] + wx*[Dx|Dxy]  ->  [v', u']
                    nc.vector.scalar_tensor_tensor(
                        uv[:], gk[:, b, 256:512], wxs, gk[:, b, 0:256],
                        op0=A.mult, op1=A.add)
                    # cols = v' + wy*u'
                    nc.vector.scalar_tensor_tensor(
                        colsP[:, b * 128:(b + 1) * 128], uv[:, 128:256], wys,
                        uv[:, 0:128], op0=A.mult, op1=A.add)
                cols = csp.tile([128, NPB], BF16)      # c-major
                for g in range(7):
                    bs_ = list(range(4 * g, min(4 * g + 4, NBLK)))
                    ptc = tpp.tile([128, 512], BF16)
                    for j, b in enumerate(bs_):
                        nc.tensor.transpose(out=ptc[:, 128 * j:128 * (j + 1)],
                                            in_=colsP[:, b * 128:(b + 1) * 128],
                                            identity=idnb[:])
                    wdt = len(bs_) * 128
                    nc.scalar.copy(out=cols[:, 512 * g:512 * g + wdt],
                                   in_=ptc[:, :wdt])
                for ch in range(7):
                    lo = 512 * ch
                    hi = min(lo + 512, NPOS)
                    nc.tensor.matmul(acc[:, lo:hi],
                                     lhsT=w_sb[:, k * 128:(k + 1) * 128],
                                     rhs=cols[:, lo:hi],
                                     start=(k == 0), stop=False)

            # ------- epilogue: += x2 on PE, then chunked ReLU + store -------
            outp = cpp.tile([128, NPOS], F32, tag="epi2")
            for ch in range(7):
                lo = 512 * ch
                hi = min(lo + 512, NPOS)
                nc.tensor.matmul(acc[:, lo:hi], lhsT=idnb[:],
                                 rhs=x2b[:, lo:hi], start=False, stop=True)
                nc.scalar.activation(outp[:, lo:hi], acc[:, lo:hi],
                                     mybir.ActivationFunctionType.Relu,
                                     bias=bias_sb[:], scale=1.0)
                nc.sync.dma_start(out=out_d[:, lo:hi], in_=outp[:, lo:hi])


def make_core_inputs(x, offset, weight, bias, x2):
    """Full inputs -> list of 8 per-core input dicts (host batch sharding)."""
    based, basew = host_consts()
    wt = np.ascontiguousarray(
        weight.reshape(128, 128, K).transpose(1, 2, 0).reshape(128, K * 128)
    ).astype(ml_dtypes.bfloat16)
    cores = []
    for i in range(N):
        off = offset[i].reshape(2 * K, NPOS).astype(np.float32)
        offd = np.ascontiguousarray(
            off.reshape(K, 2, SPT, FREE1).transpose(1, 0, 2, 3)
            .reshape(2 * K * SPT, FREE1))
        cores.append({
            "x": np.ascontiguousarray(x[i].reshape(C, NPOS), dtype=np.float32),
            "offd": offd,
            "offw": wrap_offsets(off),
            "x2": np.ascontiguousarray(
                x2[i].reshape(C, NPOS)).astype(ml_dtypes.bfloat16),
            "wt": wt,
            "bias": np.ascontiguousarray(bias.reshape(C, 1), dtype=np.float32),
            "based": based,
            "basew": basew,
        })
    return cores


_CACHED_NC = None

IN_SPECS = [("x", (C, NPOS), F32), ("offd", (2 * K * SPT, FREE1), F32),
            ("offw", (16, 2 * K * SLOTS), F32), ("x2", (C, NPOS), BF16),
            ("wt", (C, K * 128), BF16), ("bias", (C, 1), F32),
            ("based", (2 * K * SPT, FREE1), F32),
            ("basew", (16, 2 * K * SLOTS), F32)]


def _build_nc():
    global _CACHED_NC
    if _CACHED_NC is not None:
        return _CACHED_NC
    nc = bacc.Bacc("TRN2", target_bir_lowering=False, debug=False, num_devices=N)
    ins = [nc.dram_tensor(nm, list(sh), dt, kind="ExternalInput").ap()
           for nm, sh, dt in IN_SPECS]
    out = nc.dram_tensor("out", [C, NPOS], F32, kind="ExternalOutput").ap()
    with tile.TileContext(nc, trace_sim=False) as tc:
        build_kernel(tc, out, ins)
    nc.compile()
    _CACHED_NC = nc
    return nc


def run_cores(inputs, trace=False):
    """Run the SPMD kernel; returns (out [N,C,H,W] f32, exec_time_ns or None)."""
    nc = _build_nc()
    in_maps = make_core_inputs(inputs["x"], inputs["offset"], inputs["weight"],
                               inputs["bias"], inputs["x2"])
    res = bass_utils.run_bass_kernel_spmd(nc, in_maps, core_ids=list(range(N)),
                                          trace=trace)
    out = np.stack([res.results[i]["out"] for i in range(N)])
    return out.reshape(N, C, H, W), res.exec_time_ns


def kernel(x, offset, weight, bias, x2):
    x = np.asarray(x, dtype=np.float32)
    offset = np.asarray(offset, dtype=np.float32)
    weight = np.asarray(weight, dtype=np.float32)
    bias = np.asarray(bias, dtype=np.float32)
    x2 = np.asarray(x2, dtype=np.float32)
    out, _ = run_cores({"x": x, "offset": offset, "weight": weight,
                        "bias": bias, "x2": x2}, trace=False)
    return out


# revision 28
# speedup vs baseline: 1.6199x; 1.0018x over previous
"""Deformable conv2d + residual add + ReLU on 8 Trainium2 NeuronCores.

Self-contained harness entry: kernel(**inputs) -> np.ndarray.
Sharding: data-parallel over batch N=8 (one image per core); weight/bias
replicated. Each core runs the same Bass/Tile program.

v3 design (gather-descgen-bound pipeline, bf16 data path):
  Prologue (phases overlap across engines):
  A) gather-index chain on DVE directly in the SWDGE wrapped layout
     [16, (axis,k,slot)] (offsets pre-wrapped on host) -> no transposes or
     scatter DMAs; floor via x - mod(x,1). A second small chain in packed
     [126, 448] layout produces the bilinear weights, PE-transposed per
     128-position block into per-partition scalars.
  B) zero-padded image planes [x, Dy, Dx, Dxy] in bf16 built on the GpSimd
     engine (parallel with A on DVE), PE-transposed to q-major 1KB rows in
     DRAM (two 128-blocks per PSUM eviction).
  Main loop, per kernel-tap k:
  C) SWDGE dma_gather of 3200 sample rows (position-major, 2 calls),
     bilinear combine with 2 fused scalar_tensor_tensor ops per block
     ([x|Dy] + wx*[Dx|Dxy], then + wy*hi), PE transpose back to
     channel-major, bf16 matmul accumulated in PSUM.
  D) epilogue: + x2 + bias, ReLU, store f32.

Math: bilinear(x, py, px) = x[q] + wx*Dx[q] + wy*Dy[q] + wx*wy*Dxy[q] with
q = floor(py+PD)*WP + floor(px+PD) on the zero-padded grid; the zero
padding reproduces torchvision's out-of-bounds zeroing exactly, and clamping
floor() into the pad ring keeps fully-out-of-range samples at zero.
"""

import sys

for _p in ("/opt/trn_rl_repo",):
    if _p not in sys.path:
        sys.path.insert(0, _p)

import numpy as np
import ml_dtypes

import concourse.bacc as bacc
import concourse.mybir as mybir
import concourse.tile as tile
from concourse import bass_utils
from concourse.masks import make_identity

F32 = mybir.dt.float32
BF16 = mybir.dt.bfloat16
I32 = mybir.dt.int32
I16 = mybir.dt.int16
A = mybir.AluOpType

# problem constants (nn_DeformConvAddReLU2d: N=8, C=Cout=128, 56x56, 3x3)
N, C, H, W = 8, 128, 56, 56
K = 9
PD = 2
HP, WP = H + 2 * PD, W + 2 * PD          # 60, 60
Q = HP * WP                               # 3600
NPOS = H * W                              # 3136
NPB = 3200                                # samples per tap padded to 25 blocks
NBLK = NPB // 128                         # 25
ELEM = 512                                # row: [x|Dy|Dx|Dxy] x 128c bf16 (1KB)
SLOTS = NPB // 16                         # 200 wrapped idx slots per tap
SPT = 7                                   # 3136 = 7 * 448 partition packing
FREE1 = NPOS // SPT                       # 448
GSPLIT = [(0, 8), (8, 8), (16, 8), (24, 1)]   # gather call split (start, nblocks)


def host_consts():
    """Base sampling positions, pre-biased by +PD (padded-grid coords).

    Returns:
      based: [126, 448] f32 — deinterleaved packed layout (axis, k, s) x f
             for the weight chain.
      basew: [16, 2*K*SLOTS] f32 — SWDGE-wrapped layout p x (axis, k, slot)
             for the gather-index chain; padded tail positions get -1000 so
             they clamp to q=0 (a guaranteed-zero pad row).
    """
    ki = np.arange(3).repeat(3)
    kj = np.tile(np.arange(3), 3)
    i = np.arange(H)
    j = np.arange(W)
    by = (i[None, :, None] + ki[:, None, None] + 1).astype(np.float32)
    bx = (j[None, None, :] + kj[:, None, None] + 1).astype(np.float32)
    by = np.broadcast_to(by, (K, H, W)).reshape(K, NPOS)
    bx = np.broadcast_to(bx, (K, H, W)).reshape(K, NPOS)
    based = np.concatenate(
        [by.reshape(K * SPT, FREE1), bx.reshape(K * SPT, FREE1)], axis=0
    ).astype(np.float32)

    byp = np.full((K, NPB), -1000.0, dtype=np.float32)
    bxp = np.full((K, NPB), -1000.0, dtype=np.float32)
    byp[:, :NPOS] = by
    bxp[:, :NPOS] = bx
    # wrap: [K, SLOTS, 16] -> [16, K, SLOTS]
    byw = byp.reshape(K, SLOTS, 16).transpose(2, 0, 1).reshape(16, K * SLOTS)
    bxw = bxp.reshape(K, SLOTS, 16).transpose(2, 0, 1).reshape(16, K * SLOTS)
    basew = np.concatenate([byw, bxw], axis=1)
    return based, np.ascontiguousarray(basew)


def wrap_offsets(off):
    """off [2K, NPOS] f32 -> SWDGE-wrapped [16, 2*K*SLOTS] (axis, k, slot)."""
    offp = np.zeros((2 * K, NPB), dtype=np.float32)
    offp[:, :NPOS] = off
    w = offp.reshape(K, 2, SLOTS, 16).transpose(3, 1, 0, 2)  # [16, 2, K, SLOTS]
    return np.ascontiguousarray(w.reshape(16, 2 * K * SLOTS))


def build_kernel(tc, outs, ins):
    nc = tc.nc
    out_d = outs                                   # [128, NPOS] f32
    x_d, offd_d, offw_d, x2_d, wt_d, bias_d, based_d, basew_d = ins

    with tc.tile_pool(name="persist", bufs=1) as pers, \
         tc.tile_pool(name="dram", bufs=1, space="DRAM") as dp:
        g4r = dp.tile([Q, ELEM], BF16)
        wd = dp.tile([126, FREE1], F32)

        idn = pers.tile([128, 128], F32)
        make_identity(nc, idn[:])
        idnb = pers.tile([128, 128], BF16)
        nc.vector.tensor_copy(out=idnb[:], in_=idn[:])
        wsc = pers.tile([128, NBLK, 18], BF16)     # scalars: wy at k, wx at 9+k
        nc.vector.memset(wsc[:], 0.0)
        idxw = pers.tile([128, K * SLOTS], I16)    # wrapped gather indices
        w_sb = pers.tile([128, K * 128], BF16)     # lhsT per tap: [c, o]
        bias_sb = pers.tile([128, 1], F32)
        x2b = pers.tile([128, NPOS], BF16)

        nc.sync.dma_start(out=w_sb[:], in_=wt_d[:])
        nc.sync.dma_start(out=bias_sb[:], in_=bias_d[:])
        nc.sync.dma_start(out=x2b[:], in_=x2_d[:])

        # =============== Prologue: phases 1a/2/1b overlap ==================
        with tc.tile_pool(name="prosb", bufs=1) as sp, \
             tc.tile_pool(name="proev", bufs=3) as evp, \
             tc.tile_pool(name="props", bufs=2, space="PSUM") as pp, \
             tc.tile_pool(name="props2", bufs=2, space="PSUM") as pp2:
            # ---- Phase 2: padded planes in bf16, on DVE ----
            # x loads contiguously (1 packet/partition); DVE inserts padding
            # via a strided-write copy (the strided DMA was ~7K packets).
            xf = sp.tile([128, NPOS], F32, tag="xf")
            nc.sync.dma_start(out=xf[:], in_=x_d[:])
            xp = sp.tile([128, Q], F32, tag="xp")
            nc.vector.memset(xp[:], 0.0)
            xpv = xp[:].rearrange("c (h w) -> c h w", h=HP)
            nc.vector.tensor_copy(
                out=xpv[:, PD:PD + H, PD:PD + W],
                in_=xf[:].rearrange("c (h w) -> c h w", h=H))
            dxf = sp.tile([128, Q], F32, tag="dxf")    # x[q+1]-x[q] in f32
            nc.vector.memset(dxf[:, Q - 1:], 0.0)
            nc.vector.tensor_tensor(out=dxf[:, :Q - 1], in0=xp[:, 1:Q],
                                    in1=xp[:, :Q - 1], op=A.subtract)
            xpb = sp.tile([128, Q], BF16, tag="xpb")
            nc.vector.tensor_copy(out=xpb[:], in_=xp[:])
            dyb = sp.tile([128, Q], BF16, tag="dyb")
            nc.vector.memset(dyb[:, Q - WP:], 0.0)
            nc.vector.tensor_tensor(out=dyb[:, :Q - WP], in0=xp[:, WP:Q],
                                    in1=xp[:, :Q - WP], op=A.subtract)
            dxb = sp.tile([128, Q], BF16, tag="dxb")
            nc.vector.tensor_copy(out=dxb[:], in_=dxf[:])
            dxyb = sp.tile([128, Q], BF16, tag="dxyb")
            nc.vector.memset(dxyb[:, Q - WP - 1:], 0.0)
            nc.vector.tensor_tensor(out=dxyb[:, :Q - WP - 1],
                                    in0=dxf[:, WP:Q - 1],
                                    in1=dxf[:, :Q - WP - 1], op=A.subtract)
            # ---- Phase 1a: gather indices, wrapped layout, on DVE ----
            # 4 rotating [16, M] buffers (wa/wb/wc f32, wi i32) keep SBUF flat
            M = 2 * K * SLOTS                      # 3600
            wa = sp.tile([16, M], F32, tag="wa")
            wb = sp.tile([16, M], F32, tag="wb")
            wc = sp.tile([16, M], F32, tag="wc")
            wi = sp.tile([16, M], I32, tag="wi")
            nc.sync.dma_start(out=wa[:], in_=offw_d[:])    # dv
            nc.sync.dma_start(out=wb[:], in_=basew_d[:])   # base
            nc.vector.tensor_tensor(out=wc[:], in0=wa[:], in1=wb[:], op=A.add)
            nc.vector.tensor_scalar(out=wa[:], in0=wc[:], scalar1=0.0,
                                    scalar2=58.0, op0=A.max, op1=A.min)  # tcl
            nc.vector.tensor_copy(out=wi[:], in_=wa[:])    # round-to-nearest
            nc.vector.tensor_copy(out=wb[:], in_=wi[:])    # rf
            nc.vector.tensor_tensor(out=wc[:], in0=wb[:], in1=wa[:],
                                    op=A.is_gt)            # rf > tcl
            nc.vector.tensor_tensor(out=wa[:], in0=wb[:], in1=wc[:],
                                    op=A.subtract)         # floor
            qfw = sp.tile([16, K * SLOTS], F32, tag="qfw")
            nc.vector.scalar_tensor_tensor(
                out=qfw[:], in0=wa[:, :K * SLOTS], scalar=float(WP),
                in1=wa[:, K * SLOTS:], op0=A.mult, op1=A.add)
            nc.vector.tensor_copy(out=idxw[:16, :], in_=qfw[:])
            for r in (16, 32, 64):
                nc.sync.dma_start(out=idxw[r:2 * r, :], in_=idxw[0:r, :])

            planes = [xpb, dyb, dxb, dxyb]
            # two 128-blocks per PSUM tile/eviction: (0,1),(2,3),...,(28,)
            for pair in range(15):
                b0 = 2 * pair
                nb = 1 if b0 == 28 else 2
                pt = pp.tile([128, 2 * ELEM], BF16)
                for bi in range(nb):
                    b = b0 + bi
                    n = min(128, Q - b * 128)
                    for t, pl in enumerate(planes):
                        nc.tensor.transpose(
                            out=pt[:n, bi * ELEM + 128 * t:
                                   bi * ELEM + 128 * (t + 1)],
                            in_=pl[:, b * 128:b * 128 + n],
                            identity=idnb[:])
                ev = evp.tile([128, 2 * ELEM], BF16)
                rows = min(256, Q - b0 * 128)
                if nb == 2 and rows < 256:
                    # ragged pair: evict per block to keep row mapping simple
                    n0 = 128
                    n1 = rows - 128
                    nc.scalar.copy(out=ev[:n0, :ELEM], in_=pt[:n0, :ELEM])
                    nc.scalar.copy(out=ev[:n1, ELEM:], in_=pt[:n1, ELEM:])
                    nc.sync.dma_start(out=g4r[b0 * 128:b0 * 128 + n0, :],
                                      in_=ev[:n0, :ELEM])
                    nc.sync.dma_start(out=g4r[(b0 + 1) * 128:
                                              (b0 + 1) * 128 + n1, :],
                                      in_=ev[:n1, ELEM:])
                else:
                    nc.scalar.copy(out=ev[:min(rows, 128), :nb * ELEM],
                                   in_=pt[:min(rows, 128), :nb * ELEM])
                    nc.sync.dma_start(
                        out=g4r[b0 * 128:b0 * 128 + rows, :].rearrange(
                            "(b p) e -> p b e", b=nb),
                        in_=ev[:min(rows, 128), :nb * ELEM])

            # ---- Phase 1b: bilinear weights, packed layout, on DVE ----
            dv = sp.tile([126, FREE1], F32, tag="dv")
            nc.sync.dma_start(out=dv[:], in_=offd_d[:])
            bs = sp.tile([126, FREE1], F32, tag="bs")
            nc.sync.dma_start(out=bs[:], in_=based_d[:])
            tr = sp.tile([126, FREE1], F32, tag="tr")
            nc.vector.tensor_tensor(out=tr[:], in0=dv[:], in1=bs[:], op=A.add)
            tcl = sp.tile([126, FREE1], F32, tag="tcl")
            nc.vector.tensor_scalar(out=tcl[:], in0=tr[:], scalar1=0.0,
                                    scalar2=58.0, op0=A.max, op1=A.min)
            ri = sp.tile([126, FREE1], I32, tag="ri")
            nc.vector.tensor_copy(out=ri[:], in_=tcl[:])
            rf = sp.tile([126, FREE1], F32, tag="rf")
            nc.vector.tensor_copy(out=rf[:], in_=ri[:])
            gtt = sp.tile([126, FREE1], F32, tag="gtt")
            nc.vector.tensor_tensor(out=gtt[:], in0=rf[:], in1=tcl[:],
                                    op=A.is_gt)
            fl = sp.tile([126, FREE1], F32, tag="fl")
            nc.vector.tensor_tensor(out=fl[:], in0=rf[:], in1=gtt[:],
                                    op=A.subtract)
            wv = sp.tile([126, FREE1], F32, tag="wv")    # wy | wx
            nc.vector.tensor_tensor(out=wv[:], in0=tr[:], in1=fl[:],
                                    op=A.subtract)
            # reshuffle [126, 448] (a,k,s)xf -> [18, 3136] (a,k)x(s,f)
            # via DRAM (cross partition/free regrouping needs a flat hop)
            nc.sync.dma_start(out=wd[:], in_=wv[:])
            wsb2 = sp.tile([18, NPOS], F32, tag="wsb2")
            nc.sync.dma_start(
                out=wsb2[:],
                in_=wd[:].rearrange("(c s) f -> c (s f)", s=SPT))
            for b in range(NBLK):
                n = min(128, NPOS - b * 128)
                if n <= 0:
                    break
                ptw = pp2.tile([128, 32], F32)
                nc.tensor.transpose(out=ptw[:n, 0:18],
                                    in_=wsb2[:, b * 128:b * 128 + n],
                                    identity=idn[:18, :18])
                nc.scalar.copy(out=wsc[:n, b, :], in_=ptw[:n, 0:18])

        # ---------------- Phase 3: gather / combine / matmul ----------------
        with tc.tile_pool(name="gk", bufs=3) as gp, \
             tc.tile_pool(name="cp", bufs=2) as cpp, \
             tc.tile_pool(name="cols", bufs=2) as csp, \
             tc.tile_pool(name="uv", bufs=4) as uvp, \
             tc.tile_pool(name="accp", bufs=1, space="PSUM") as accp, \
             tc.tile_pool(name="tps", bufs=1, space="PSUM") as tpp:
            acc = accp.tile([128, NPOS], F32)
            for k in range(K):
                gk = gp.tile([128, NBLK, ELEM], BF16)
                for hb, nb in GSPLIT:
                    nc.gpsimd.dma_gather(
                        gk[:, hb:hb + nb, :], g4r[:],
                        idxw[:, k * SLOTS + hb * 8:k * SLOTS + (hb + nb) * 8],
                        num_idxs=nb * 128, num_idxs_reg=nb * 128,
                        elem_size=ELEM)
                colsP = cpp.tile([128, NPB], BF16)     # pos-major combined
                for b in range(NBLK):
                    wys = wsc[:, b, k:k + 1]
                    wxs = wsc[:, b, 9 + k:10 + k]
                    uv = uvp.tile([128, 256], BF16, tag="uv")
                    # uv = [x|Dy] + wx*[Dx|Dxy]  ->  [v', u']
                    nc.vector.scalar_tensor_tensor(
                        uv[:], gk[:, b, 256:512], wxs, gk[:, b, 0:256],
                        op0=A.mult, op1=A.add)
                    # cols = v' + wy*u'
                    nc.vector.scalar_tensor_tensor(
                        colsP[:, b * 128:(b + 1) * 128], uv[:, 128:256], wys,
                        uv[:, 0:128], op0=A.mult, op1=A.add)
                cols = csp.tile([128, NPB], BF16)      # c-major
                for g in range(7):
                    bs_ = list(range(4 * g, min(4 * g + 4, NBLK)))
                    ptc = tpp.tile([128, 512], BF16)
                    for j, b in enumerate(bs_):
                        nc.tensor.transpose(out=ptc[:, 128 * j:128 * (j + 1)],
                                            in_=colsP[:, b * 128:(b + 1) * 128],
                                            identity=idnb[:])
                    wdt = len(bs_) * 128
                    nc.scalar.copy(out=cols[:, 512 * g:512 * g + wdt],
                                   in_=ptc[:, :wdt])
                for ch in range(7):
                    lo = 512 * ch
                    hi = min(lo + 512, NPOS)
                    nc.tensor.matmul(acc[:, lo:hi],
                                     lhsT=w_sb[:, k * 128:(k + 1) * 128],
                                     rhs=cols[:, lo:hi],
                                     start=(k == 0), stop=False)

            # ------- epilogue: += x2 on PE, then chunked ReLU + store -------
            outp = cpp.tile([128, NPOS], F32, tag="epi2")
            for ch in range(7):
                lo = 512 * ch
                hi = min(lo + 512, NPOS)
                nc.tensor.matmul(acc[:, lo:hi], lhsT=idnb[:],
                                 rhs=x2b[:, lo:hi], start=False, stop=True)
                nc.scalar.activation(outp[:, lo:hi], acc[:, lo:hi],
                                     mybir.ActivationFunctionType.Relu,
                                     bias=bias_sb[:], scale=1.0)
                nc.sync.dma_start(out=out_d[:, lo:hi], in_=outp[:, lo:hi])


def make_core_inputs(x, offset, weight, bias, x2):
    """Full inputs -> list of 8 per-core input dicts (host batch sharding)."""
    based, basew = host_consts()
    wt = np.ascontiguousarray(
        weight.reshape(128, 128, K).transpose(1, 2, 0).reshape(128, K * 128)
    ).astype(ml_dtypes.bfloat16)
    cores = []
    for i in range(N):
        off = offset[i].reshape(2 * K, NPOS).astype(np.float32)
        offd = np.ascontiguousarray(
            off.reshape(K, 2, SPT, FREE1).transpose(1, 0, 2, 3)
            .reshape(2 * K * SPT, FREE1))
        cores.append({
            "x": np.ascontiguousarray(x[i].reshape(C, NPOS), dtype=np.float32),
            "offd": offd,
            "offw": wrap_offsets(off),
            "x2": np.ascontiguousarray(
                x2[i].reshape(C, NPOS)).astype(ml_dtypes.bfloat16),
            "wt": wt,
            "bias": np.ascontiguousarray(bias.reshape(C, 1), dtype=np.float32),
            "based": based,
            "basew": basew,
        })
    return cores


_CACHED_NC = None

IN_SPECS = [("x", (C, NPOS), F32), ("offd", (2 * K * SPT, FREE1), F32),
            ("offw", (16, 2 * K * SLOTS), F32), ("x2", (C, NPOS), BF16),
            ("wt", (C, K * 128), BF16), ("bias", (C, 1), F32),
            ("based", (2 * K * SPT, FREE1), F32),
            ("basew", (16, 2 * K * SLOTS), F32)]


def _build_nc():
    global _CACHED_NC
    if _CACHED_NC is not None:
        return _CACHED_NC
    nc = bacc.Bacc("TRN2", target_bir_lowering=False, debug=False, num_devices=N)
    ins = [nc.dram_tensor(nm, list(sh), dt, kind="ExternalInput").ap()
           for nm, sh, dt in IN_SPECS]
    out = nc.dram_tensor("out", [C, NPOS], F32, kind="ExternalOutput").ap()
    with tile.TileContext(nc, trace_sim=False) as tc:
        build_kernel(tc, out, ins)
    nc.compile()
    _CACHED_NC = nc
    return nc


def run_cores(inputs, trace=False):
    """Run the SPMD kernel; returns (out [N,C,H,W] f32, exec_time_ns or None)."""
    nc = _build_nc()
    in_maps = make_core_inputs(inputs["x"], inputs["offset"], inputs["weight"],
                               inputs["bias"], inputs["x2"])
    res = bass_utils.run_bass_kernel_spmd(nc, in_maps, core_ids=list(range(N)),
                                          trace=trace)
    out = np.stack([res.results[i]["out"] for i in range(N)])
    return out.reshape(N, C, H, W), res.exec_time_ns


def kernel(x, offset, weight, bias, x2):
    x = np.asarray(x, dtype=np.float32)
    offset = np.asarray(offset, dtype=np.float32)
    weight = np.asarray(weight, dtype=np.float32)
    bias = np.asarray(bias, dtype=np.float32)
    x2 = np.asarray(x2, dtype=np.float32)
    out, _ = run_cores({"x": x, "offset": offset, "weight": weight,
                        "bias": bias, "x2": x2}, trace=False)
    return out


# revision 30
# speedup vs baseline: 1.6684x; 1.0299x over previous
"""Deformable conv2d + residual add + ReLU on 8 Trainium2 NeuronCores.

Self-contained harness entry: kernel(**inputs) -> np.ndarray.
Sharding: data-parallel over batch N=8 (one image per core); weight/bias
replicated. Each core runs the same Bass/Tile program.

Design (SWDGE-descgen-bound pipeline, bf16 data path):
  Prologue (ordered so PE transpose work overlaps DVE index math):
  A) zero-padded image planes [x, Dy, Dx, Dxy] in bf16 on DVE (x loaded
     contiguously, padding inserted by a strided-write copy), PE-transposed
     to q-major 1KB rows in DRAM (two 128-blocks per PSUM eviction).
  B) gather-index chain on DVE directly in the SWDGE wrapped layout
     [16, (axis,k,slot)] (offsets pre-wrapped on host) -> no transposes or
     scatter DMAs; floor via round-to-nearest + is_gt correction. A second
     small chain in packed [126, 448] layout produces the bilinear weights,
     PE-transposed per 128-position block into per-partition scalars.
  Main loop, per kernel-tap k:
  C) SWDGE dma_gather of 3200 sample rows (position-major; 4 calls of
     <=1024 idxs each -- the SWDGE ring holds ~65 descriptors/ring, larger
     calls deadlock), bilinear combine with 2 fused scalar_tensor_tensor
     ops per block ([x|Dy] + wx*[Dx|Dxy], then + wy*hi), PE transpose back
     to channel-major, bf16 matmul accumulated in PSUM (3-deep gather
     buffering to keep descgen back-to-back).
  D) epilogue: x2 added in PSUM via identity matmul (bf16), then per-512-col
     chunk: ReLU+bias on ACT, store f32.

Math: bilinear(x, py, px) = x[q] + wx*Dx[q] + wy*Dy[q] + wx*wy*Dxy[q] with
q = floor(py+PD)*WP + floor(px+PD) on the zero-padded grid; the zero
padding reproduces torchvision's out-of-bounds zeroing exactly, and clamping
floor() into the pad ring keeps fully-out-of-range samples at zero.
"""

import sys

for _p in ("/opt/trn_rl_repo",):
    if _p not in sys.path:
        sys.path.insert(0, _p)

import numpy as np
import ml_dtypes

import concourse.bacc as bacc
import concourse.mybir as mybir
import concourse.tile as tile
from concourse import bass_utils
from concourse.masks import make_identity

F32 = mybir.dt.float32
BF16 = mybir.dt.bfloat16
I32 = mybir.dt.int32
I16 = mybir.dt.int16
A = mybir.AluOpType

# problem constants (nn_DeformConvAddReLU2d: N=8, C=Cout=128, 56x56, 3x3)
N, C, H, W = 8, 128, 56, 56
K = 9
PD = 2
HP, WP = H + 2 * PD, W + 2 * PD          # 60, 60
Q = HP * WP                               # 3600
NPOS = H * W                              # 3136
NPB = 3200                                # samples per tap padded to 25 blocks
NBLK = NPB // 128                         # 25
ELEM = 512                                # row: [x|Dy|Dx|Dxy] x 128c bf16 (1KB)
SLOTS = NPB // 16                         # 200 wrapped idx slots per tap
SPT = 7                                   # 3136 = 7 * 448 partition packing
FREE1 = NPOS // SPT                       # 448
GSPLIT = [(0, 8), (8, 8), (16, 8), (24, 1)]   # gather call split (start, nblocks)


def host_consts():
    """Base sampling positions, pre-biased by +PD (padded-grid coords).

    Returns:
      based: [126, 448] f32 — deinterleaved packed layout (axis, k, s) x f
             for the weight chain.
      basew: [16, 2*K*SLOTS] f32 — SWDGE-wrapped layout p x (axis, k, slot)
             for the gather-index chain; padded tail positions get -1000 so
             they clamp to q=0 (a guaranteed-zero pad row).
    """
    ki = np.arange(3).repeat(3)
    kj = np.tile(np.arange(3), 3)
    i = np.arange(H)
    j = np.arange(W)
    by = (i[None, :, None] + ki[:, None, None] + 1).astype(np.float32)
    bx = (j[None, None, :] + kj[:, None, None] + 1).astype(np.float32)
    by = np.broadcast_to(by, (K, H, W)).reshape(K, NPOS)
    bx = np.broadcast_to(bx, (K, H, W)).reshape(K, NPOS)
    based = np.concatenate(
        [by.reshape(K * SPT, FREE1), bx.reshape(K * SPT, FREE1)], axis=0
    ).astype(np.float32)

    byp = np.full((K, NPB), -1000.0, dtype=np.float32)
    bxp = np.full((K, NPB), -1000.0, dtype=np.float32)
    byp[:, :NPOS] = by
    bxp[:, :NPOS] = bx
    # wrap: [K, SLOTS, 16] -> [16, K, SLOTS]
    byw = byp.reshape(K, SLOTS, 16).transpose(2, 0, 1).reshape(16, K * SLOTS)
    bxw = bxp.reshape(K, SLOTS, 16).transpose(2, 0, 1).reshape(16, K * SLOTS)
    basew = np.concatenate([byw, bxw], axis=1)
    return based, np.ascontiguousarray(basew)


def wrap_offsets(off):
    """off [2K, NPOS] f32 -> SWDGE-wrapped [16, 2*K*SLOTS] (axis, k, slot)."""
    offp = np.zeros((2 * K, NPB), dtype=np.float32)
    offp[:, :NPOS] = off
    w = offp.reshape(K, 2, SLOTS, 16).transpose(3, 1, 0, 2)  # [16, 2, K, SLOTS]
    return np.ascontiguousarray(w.reshape(16, 2 * K * SLOTS))


def build_kernel(tc, outs, ins):
    nc = tc.nc
    out_d = outs                                   # [128, NPOS] f32
    x_d, offd_d, offw_d, x2_d, wt_d, bias_d, based_d, basew_d = ins

    with tc.tile_pool(name="persist", bufs=1) as pers, \
         tc.tile_pool(name="dram", bufs=1, space="DRAM") as dp:
        g4r = dp.tile([Q, ELEM], BF16)
        wd = dp.tile([126, FREE1], F32)

        idn = pers.tile([128, 128], F32)
        make_identity(nc, idn[:])
        idnb = pers.tile([128, 128], BF16)
        nc.vector.tensor_copy(out=idnb[:], in_=idn[:])
        wsc = pers.tile([128, NBLK, 18], BF16)     # scalars: wy at k, wx at 9+k
        nc.vector.memset(wsc[:], 0.0)
        idxw = pers.tile([128, K * SLOTS], I16)    # wrapped gather indices
        w_sb = pers.tile([128, K * 128], BF16)     # lhsT per tap: [c, o]
        bias_sb = pers.tile([128, 1], F32)
        x2b = pers.tile([128, NPOS], BF16)

        nc.sync.dma_start(out=w_sb[:], in_=wt_d[:])
        nc.sync.dma_start(out=bias_sb[:], in_=bias_d[:])
        nc.sync.dma_start(out=x2b[:], in_=x2_d[:])

        # =============== Prologue: phases 1a/2/1b overlap ==================
        with tc.tile_pool(name="prosb", bufs=1) as sp, \
             tc.tile_pool(name="proev", bufs=3) as evp, \
             tc.tile_pool(name="props", bufs=2, space="PSUM") as pp, \
             tc.tile_pool(name="props2", bufs=2, space="PSUM") as pp2:
            # ---- Phase 2: padded planes in bf16, on DVE ----
            # x loads contiguously (1 packet/partition); DVE inserts padding
            # via a strided-write copy (the strided DMA was ~7K packets).
            xf = sp.tile([128, NPOS], F32, tag="xf")
            nc.sync.dma_start(out=xf[:], in_=x_d[:])
            xp = sp.tile([128, Q], F32, tag="xp")
            nc.vector.memset(xp[:], 0.0)
            xpv = xp[:].rearrange("c (h w) -> c h w", h=HP)
            nc.vector.tensor_copy(
                out=xpv[:, PD:PD + H, PD:PD + W],
                in_=xf[:].rearrange("c (h w) -> c h w", h=H))
            xpb = sp.tile([128, Q], BF16, tag="xpb")
            nc.vector.tensor_copy(out=xpb[:], in_=xp[:])
            dxb = sp.tile([128, Q], BF16, tag="dxb")
            nc.vector.memset(dxb[:, Q - 1:], 0.0)
            nc.vector.tensor_tensor(out=dxb[:, :Q - 1], in0=xp[:, 1:Q],
                                    in1=xp[:, :Q - 1], op=A.subtract)
            dyb = sp.tile([128, Q], BF16, tag="dyb")
            nc.vector.memset(dyb[:, Q - WP:], 0.0)
            nc.vector.tensor_tensor(out=dyb[:, :Q - WP], in0=xp[:, WP:Q],
                                    in1=xp[:, :Q - WP], op=A.subtract)
            dxyb = sp.tile([128, Q], BF16, tag="dxyb")
            nc.vector.memset(dxyb[:, Q - WP - 1:], 0.0)
            nc.vector.tensor_tensor(out=dxyb[:, :Q - WP - 1],
                                    in0=dxb[:, WP:Q - 1],
                                    in1=dxb[:, :Q - WP - 1], op=A.subtract)
            # ---- Phase 1a: gather indices, wrapped layout, on DVE ----
            # 4 rotating [16, M] buffers (wa/wb/wc f32, wi i32) keep SBUF flat
            M = 2 * K * SLOTS                      # 3600
            wa = sp.tile([16, M], F32, tag="wa")
            wb = sp.tile([16, M], F32, tag="wb")
            wc = sp.tile([16, M], F32, tag="wc")
            wi = sp.tile([16, M], I32, tag="wi")
            nc.sync.dma_start(out=wa[:], in_=offw_d[:])    # dv
            nc.sync.dma_start(out=wb[:], in_=basew_d[:])   # base
            nc.vector.tensor_tensor(out=wc[:], in0=wa[:], in1=wb[:], op=A.add)
            nc.vector.tensor_scalar(out=wa[:], in0=wc[:], scalar1=0.0,
                                    scalar2=58.0, op0=A.max, op1=A.min)  # tcl
            nc.vector.tensor_copy(out=wi[:], in_=wa[:])    # round-to-nearest
            nc.vector.tensor_copy(out=wb[:], in_=wi[:])    # rf
            nc.vector.tensor_tensor(out=wc[:], in0=wb[:], in1=wa[:],
                                    op=A.is_gt)            # rf > tcl
            nc.vector.tensor_tensor(out=wa[:], in0=wb[:], in1=wc[:],
                                    op=A.subtract)         # floor
            qfw = sp.tile([16, K * SLOTS], F32, tag="qfw")
            nc.vector.scalar_tensor_tensor(
                out=qfw[:], in0=wa[:, :K * SLOTS], scalar=float(WP),
                in1=wa[:, K * SLOTS:], op0=A.mult, op1=A.add)
            nc.vector.tensor_copy(out=idxw[:16, :], in_=qfw[:])
            for r in (16, 32, 64):
                nc.sync.dma_start(out=idxw[r:2 * r, :], in_=idxw[0:r, :])

            planes = [xpb, dyb, dxb, dxyb]
            # two 128-blocks per PSUM tile/eviction: (0,1),(2,3),...,(28,)
            for pair in range(15):
                b0 = 2 * pair
                nb = 1 if b0 == 28 else 2
                pt = pp.tile([128, 2 * ELEM], BF16)
                for bi in range(nb):
                    b = b0 + bi
                    n = min(128, Q - b * 128)
                    for t, pl in enumerate(planes):
                        nc.tensor.transpose(
                            out=pt[:n, bi * ELEM + 128 * t:
                                   bi * ELEM + 128 * (t + 1)],
                            in_=pl[:, b * 128:b * 128 + n],
                            identity=idnb[:])
                ev = evp.tile([128, 2 * ELEM], BF16)
                rows = min(256, Q - b0 * 128)
                if nb == 2 and rows < 256:
                    # ragged pair: evict per block to keep row mapping simple
                    n0 = 128
                    n1 = rows - 128
                    nc.scalar.copy(out=ev[:n0, :ELEM], in_=pt[:n0, :ELEM])
                    nc.scalar.copy(out=ev[:n1, ELEM:], in_=pt[:n1, ELEM:])
                    nc.sync.dma_start(out=g4r[b0 * 128:b0 * 128 + n0, :],
                                      in_=ev[:n0, :ELEM])
                    nc.sync.dma_start(out=g4r[(b0 + 1) * 128:
                                              (b0 + 1) * 128 + n1, :],
                                      in_=ev[:n1, ELEM:])
                else:
                    nc.scalar.copy(out=ev[:min(rows, 128), :nb * ELEM],
                                   in_=pt[:min(rows, 128), :nb * ELEM])
                    nc.sync.dma_start(
                        out=g4r[b0 * 128:b0 * 128 + rows, :].rearrange(
                            "(b p) e -> p b e", b=nb),
                        in_=ev[:min(rows, 128), :nb * ELEM])

            # ---- Phase 1b: bilinear weights, packed layout, on DVE ----
            dv = sp.tile([126, FREE1], F32, tag="dv")
            nc.sync.dma_start(out=dv[:], in_=offd_d[:])
            bs = sp.tile([126, FREE1], F32, tag="bs")
            nc.sync.dma_start(out=bs[:], in_=based_d[:])
            tr = sp.tile([126, FREE1], F32, tag="tr")
            nc.vector.tensor_tensor(out=tr[:], in0=dv[:], in1=bs[:], op=A.add)
            tcl = sp.tile([126, FREE1], F32, tag="tcl")
            nc.vector.tensor_scalar(out=tcl[:], in0=tr[:], scalar1=0.0,
                                    scalar2=58.0, op0=A.max, op1=A.min)
            ri = sp.tile([126, FREE1], I32, tag="ri")
            nc.vector.tensor_copy(out=ri[:], in_=tcl[:])
            rf = sp.tile([126, FREE1], F32, tag="rf")
            nc.vector.tensor_copy(out=rf[:], in_=ri[:])
            gtt = sp.tile([126, FREE1], F32, tag="gtt")
            nc.vector.tensor_tensor(out=gtt[:], in0=rf[:], in1=tcl[:],
                                    op=A.is_gt)
            fl = sp.tile([126, FREE1], F32, tag="fl")
            nc.vector.tensor_tensor(out=fl[:], in0=rf[:], in1=gtt[:],
                                    op=A.subtract)
            wv = sp.tile([126, FREE1], F32, tag="wv")    # wy | wx
            nc.vector.tensor_tensor(out=wv[:], in0=tr[:], in1=fl[:],
                                    op=A.subtract)
            # reshuffle [126, 448] (a,k,s)xf -> [18, 3136] (a,k)x(s,f)
            # via DRAM (cross partition/free regrouping needs a flat hop)
            nc.sync.dma_start(out=wd[:], in_=wv[:])
            wsb2 = sp.tile([18, NPOS], F32, tag="wsb2")
            nc.sync.dma_start(
                out=wsb2[:],
                in_=wd[:].rearrange("(c s) f -> c (s f)", s=SPT))
            for b in range(NBLK):
                n = min(128, NPOS - b * 128)
                if n <= 0:
                    break
                ptw = pp2.tile([128, 32], F32)
                nc.tensor.transpose(out=ptw[:n, 0:18],
                                    in_=wsb2[:, b * 128:b * 128 + n],
                                    identity=idn[:18, :18])
                nc.scalar.copy(out=wsc[:n, b, :], in_=ptw[:n, 0:18])

        # ---------------- Phase 3: gather / combine / matmul ----------------
        with tc.tile_pool(name="gk", bufs=3) as gp, \
             tc.tile_pool(name="cp", bufs=2) as cpp, \
             tc.tile_pool(name="cols", bufs=2) as csp, \
             tc.tile_pool(name="uv", bufs=4) as uvp, \
             tc.tile_pool(name="accp", bufs=1, space="PSUM") as accp, \
             tc.tile_pool(name="tps", bufs=1, space="PSUM") as tpp:
            acc = accp.tile([128, NPOS], F32)
            # residual x2 seeds the PSUM accumulation (runs while PE is idle
            # during the first gather's descgen)
            for ch in range(7):
                lo = 512 * ch
                hi = min(lo + 512, NPOS)
                nc.tensor.matmul(acc[:, lo:hi], lhsT=idnb[:],
                                 rhs=x2b[:, lo:hi], start=True, stop=False)
            for k in range(K):
                gk = gp.tile([128, NBLK, ELEM], BF16)
                for hb, nb in GSPLIT:
                    nc.gpsimd.dma_gather(
                        gk[:, hb:hb + nb, :], g4r[:],
                        idxw[:, k * SLOTS + hb * 8:k * SLOTS + (hb + nb) * 8],
                        num_idxs=nb * 128, num_idxs_reg=nb * 128,
                        elem_size=ELEM)
                colsP = cpp.tile([128, NPB], BF16)     # pos-major combined
                for b in range(NBLK):
                    wys = wsc[:, b, k:k + 1]
                    wxs = wsc[:, b, 9 + k:10 + k]
                    uv = uvp.tile([128, 256], BF16, tag="uv")
                    # uv = [x|Dy] + wx*[Dx|Dxy]  ->  [v', u']
                    nc.vector.scalar_tensor_tensor(
                        uv[:], gk[:, b, 256:512], wxs, gk[:, b, 0:256],
                        op0=A.mult, op1=A.add)
                    # cols = v' + wy*u'
                    nc.vector.scalar_tensor_tensor(
                        colsP[:, b * 128:(b + 1) * 128], uv[:, 128:256], wys,
                        uv[:, 0:128], op0=A.mult, op1=A.add)
                cols = csp.tile([128, NPB], BF16)      # c-major
                for g in range(7):
                    bs_ = list(range(4 * g, min(4 * g + 4, NBLK)))
                    ptc = tpp.tile([128, 512], BF16)
                    for j, b in enumerate(bs_):
                        nc.tensor.transpose(out=ptc[:, 128 * j:128 * (j + 1)],
                                            in_=colsP[:, b * 128:(b + 1) * 128],
                                            identity=idnb[:])
                    wdt = len(bs_) * 128
                    nc.scalar.copy(out=cols[:, 512 * g:512 * g + wdt],
                                   in_=ptc[:, :wdt])
                for ch in range(7):
                    lo = 512 * ch
                    hi = min(lo + 512, NPOS)
                    nc.tensor.matmul(acc[:, lo:hi],
                                     lhsT=w_sb[:, k * 128:(k + 1) * 128],
                                     rhs=cols[:, lo:hi],
                                     start=False, stop=(k == K - 1))

            # ------- epilogue: chunked ReLU + store -------
            outp = cpp.tile([128, NPOS], F32, tag="epi2")
            for ch in range(7):
                lo = 512 * ch
                hi = min(lo + 512, NPOS)
                nc.scalar.activation(outp[:, lo:hi], acc[:, lo:hi],
                                     mybir.ActivationFunctionType.Relu,
                                     bias=bias_sb[:], scale=1.0)
                nc.sync.dma_start(out=out_d[:, lo:hi], in_=outp[:, lo:hi])


def make_core_inputs(x, offset, weight, bias, x2):
    """Full inputs -> list of 8 per-core input dicts (host batch sharding)."""
    based, basew = host_consts()
    wt = np.ascontiguousarray(
        weight.reshape(128, 128, K).transpose(1, 2, 0).reshape(128, K * 128)
    ).astype(ml_dtypes.bfloat16)
    cores = []
    for i in range(N):
        off = offset[i].reshape(2 * K, NPOS).astype(np.float32)
        offd = np.ascontiguousarray(
            off.reshape(K, 2, SPT, FREE1).transpose(1, 0, 2, 3)
            .reshape(2 * K * SPT, FREE1))
        cores.append({
            "x": np.ascontiguousarray(x[i].reshape(C, NPOS), dtype=np.float32),
            "offd": offd,
            "offw": wrap_offsets(off),
            "x2": np.ascontiguousarray(
                x2[i].reshape(C, NPOS)).astype(ml_dtypes.bfloat16),
            "wt": wt,
            "bias": np.ascontiguousarray(bias.reshape(C, 1), dtype=np.float32),
            "based": based,
            "basew": basew,
        })
    return cores


_CACHED_NC = None

IN_SPECS = [("x", (C, NPOS), F32), ("offd", (2 * K * SPT, FREE1), F32),
            ("offw", (16, 2 * K * SLOTS), F32), ("x2", (C, NPOS), BF16),
            ("wt", (C, K * 128), BF16), ("bias", (C, 1), F32),
            ("based", (2 * K * SPT, FREE1), F32),
            ("basew", (16, 2 * K * SLOTS), F32)]


def _build_nc():
    global _CACHED_NC
    if _CACHED_NC is not None:
        return _CACHED_NC
    nc = bacc.Bacc("TRN2", target_bir_lowering=False, debug=False, num_devices=N)
    ins = [nc.dram_tensor(nm, list(sh), dt, kind="ExternalInput").ap()
           for nm, sh, dt in IN_SPECS]
    out = nc.dram_tensor("out", [C, NPOS], F32, kind="ExternalOutput").ap()
    with tile.TileContext(nc, trace_sim=False) as tc:
        build_kernel(tc, out, ins)
    nc.compile()
    _CACHED_NC = nc
    return nc


def run_cores(inputs, trace=False):
    """Run the SPMD kernel; returns (out [N,C,H,W] f32, exec_time_ns or None)."""
    nc = _build_nc()
    in_maps = make_core_inputs(inputs["x"], inputs["offset"], inputs["weight"],
                               inputs["bias"], inputs["x2"])
    res = bass_utils.run_bass_kernel_spmd(nc, in_maps, core_ids=list(range(N)),
                                          trace=trace)
    out = np.stack([res.results[i]["out"] for i in range(N)])
    return out.reshape(N, C, H, W), res.exec_time_ns


def kernel(x, offset, weight, bias, x2):
    x = np.asarray(x, dtype=np.float32)
    offset = np.asarray(offset, dtype=np.float32)
    weight = np.asarray(weight, dtype=np.float32)
    bias = np.asarray(bias, dtype=np.float32)
    x2 = np.asarray(x2, dtype=np.float32)
    out, _ = run_cores({"x": x, "offset": offset, "weight": weight,
                        "bias": bias, "x2": x2}, trace=False)
    return out
